# revision 34
# baseline (speedup 1.0000x reference)
"""Trainium2 Bass kernel for nn_DecoderLayer (dense transformer decoder layer).

Strategy (8 NeuronCores, full inputs in / full output out):
  - core c handles batch b = c//4 and query-quarter r = c%4 (rows [r*S/4, (r+1)*S/4)).
  - All matmul operands are bf16 (weights cast host-side, activations cast at
    the PSUM->SBUF copy points); PSUM accumulation stays fp32, as do residuals
    and LayerNorm statistics.
  - K^T and V(+ones column) live entirely in SBUF (no DRAM round-trip); x^T is
    loaded to SBUF once per block and sliced for the K/V/Q projections.
  - Attention per head: S^T[k, q] scores via PE (64-partition contraction),
    exp on ACT (1/8 scale fused, bf16 out), causal mask as multiplicative
    bf16 [128, W] tiles, softmax denominators free via a ones column in V,
    normalization deferred: unnormalized head outputs + per-head denominator
    rows are copied out of PSUM, one batched reciprocal [H, W] per block, then
    per-head PE ones-broadcast + vector multiply.
  - The single collective: bf16 AllGather of x1 (post-LN1) within each 4-core
    batch group; the Q2 projection is issued after it so it overlaps.
  - LayerNorm in transposed layout: cross-partition sums via ones-matmul on
    the PE, stats broadcast back to [128, W] via ones-matmul.
"""

import sys

if "/opt/trn_rl_repo" not in sys.path:
    sys.path.insert(0, "/opt/trn_rl_repo")

import numpy as np

P = 128
HD = 64
HD1 = HD + 1
EPS = 1e-5


class Cfg:
    def __init__(self, B=2, S=2048, D=1024, H=16, DFF=4096, use_collective=True,
                 fake_gather=False):
        self.B, self.S, self.D, self.H, self.DFF = B, S, D, H, DFF
        self.fake_gather = fake_gather
        self.W = S // 4            # local query rows per core
        self.DT = D // P           # feature-dim tiles
        self.NT = S // P           # sequence tiles (keys)
        self.FT = DFF // P         # ffn hidden tiles
        self.HP = P // HD          # heads per partition-tile (2)
        self.NCH = max(1, S // 512)   # n-chunks for K-orientation matmuls
        self.NCW = S // self.NCH      # n-chunk width (<=512)
        self.VCW = min(512, D)        # v-dout chunk width
        self.VCN = D // self.VCW
        self.KTG = 2                  # k-tiles per exp group
        self.NG = self.NT // self.KTG
        self.use_collective = use_collective
        assert D == H * HD
        assert self.W % P == 0 and D % P == 0 and DFF % P == 0 and S % P == 0
        assert self.NT % self.KTG == 0


class Flags:
    def __init__(self):
        self.qkb1 = self.vb1 = self.ob1 = False
        self.qkb2 = self.vb2 = self.ob2 = False
        self.fb1 = self.fb2 = False
        self.g1 = self.b1 = self.g2 = self.b2 = self.g3 = self.b3 = False
        self.m1 = True      # trg mask multiplicative tiles
        self.kb2 = False    # enc mask additive per-k bias
        self.causal = False  # trg mask is lower-triangular -> column skip


def _build(nc, tc, cfg, fl):
    import concourse.bass as bass
    import concourse.mybir as mybir
    import concourse.tile as tile  # noqa: F401
    from contextlib import ExitStack

    AF = mybir.ActivationFunctionType
    f32 = mybir.dt.float32
    bf16 = mybir.dt.bfloat16

    B, S, D, H, DFF = cfg.B, cfg.S, cfg.D, cfg.H, cfg.DFF
    W, DT, NT, FT, HP = cfg.W, cfg.DT, cfg.NT, cfg.FT, cfg.HP
    NCH, NCW, VCW, VCN = cfg.NCH, cfg.NCW, cfg.VCW, cfg.VCN
    KTG, NG = cfg.KTG, cfg.NG
    HPC = VCW // HD  # heads per v-chunk

    # ---------------- DRAM parameters ----------------
    def din(name, shape, dt=f32):
        return nc.dram_tensor(name, shape, dt, kind="ExternalInput").ap()

    xTqb = din("xTqb", [D, W], bf16)     # natural token quarter (K/V source)
    xTlb = din("xTlb", [D, W], bf16)     # interleaved queries, bf16 (Q source)
    xTl = din("xTl", [D, W])             # interleaved queries, fp32 (residual)
    qkvwT1 = din("qkvwT1", [D, 3 * D], bf16)
    qkvwT2 = din("qkvwT2", [D, 3 * D], bf16)
    owT1 = din("owT1", [D, D], bf16)
    owT2 = din("owT2", [D, D], bf16)
    w1T = din("w1T", [D, DFF], bf16)
    w2T = din("w2T", [DFF, D], bf16)
    m1 = din("m1", [NT, P, W], bf16) if fl.m1 else None
    kb2 = din("kb2", [NT, P, 1]) if fl.kb2 else None
    qkvb1 = din("qkvb1", [3 * D]) if fl.qkb1 else None
    qkvb2 = din("qkvb2", [3 * D]) if fl.qkb2 else None
    vb1 = din("vb1", [P, D]) if fl.vb1 else None
    vb2 = din("vb2", [P, D]) if fl.vb2 else None
    ob1 = din("ob1", [D]) if fl.ob1 else None
    ob2 = din("ob2", [D]) if fl.ob2 else None
    fb1d = din("fb1", [DFF]) if fl.fb1 else None
    fb2d = din("fb2", [D]) if fl.fb2 else None
    lnp = {}
    for nm, use in [("g1", fl.g1), ("b1", fl.b1), ("g2", fl.g2),
                    ("b2", fl.b2), ("g3", fl.g3), ("b3", fl.b3)]:
        lnp[nm] = din(nm, [D]) if use else None
    out = nc.dram_tensor("out", [D, W], f32, kind="ExternalOutput").ap()

    NTQ = W // P          # token tiles in the local quarter
    KL = D * W            # K staging elements
    VL = NTQ * P * H * HD1  # V staging elements
    HX = H * HD1

    es = ExitStack()
    with es:
        dramp = es.enter_context(tc.tile_pool(name="dram", bufs=1, space="DRAM"))
        kloc1 = dramp.tile([KL], bf16)
        kg1 = dramp.tile([4 * KL], bf16)
        vloc1 = dramp.tile([VL], bf16)
        vg1 = dramp.tile([4 * VL], bf16)
        kloc2 = dramp.tile([KL], bf16)
        kg2 = dramp.tile([4 * KL], bf16)
        vloc2 = dramp.tile([VL], bf16)
        vg2 = dramp.tile([4 * VL], bf16)

        const = es.enter_context(tc.tile_pool(name="const", bufs=1))
        ones_p1 = const.tile([P, 1], f32)
        nc.vector.memset(ones_p1[:, :], 1.0)
        ones_1p = const.tile([1, P], f32)
        nc.vector.memset(ones_1p[0:1, :], 1.0)
        ones_hd = const.tile([P, HD], f32)
        nc.vector.memset(ones_hd[:, :], 1.0)
        eps_t = const.tile([1, 1], f32)
        nc.vector.memset(eps_t[0:1, :], EPS)

        def ldvec(dram_vec, n_tiles, name):
            """[D]-style vector -> [P, n_tiles] sbuf tile (per-partition slices)."""
            t = const.tile([P, n_tiles], f32, tag=name)
            nc.sync.dma_start(
                out=t[:, :],
                in_=dram_vec.rearrange("(t p) -> p t", p=P),
            )
            return t

        qkb1sb = ldvec(qkvb1[0 : 2 * D], 2 * DT, "qkb1") if fl.qkb1 else None
        qkb2sb = ldvec(qkvb2[0 : 2 * D], 2 * DT, "qkb2") if fl.qkb2 else None
        ob1sb = ldvec(ob1, DT, "ob1") if fl.ob1 else None
        ob2sb = ldvec(ob2, DT, "ob2") if fl.ob2 else None
        fb1sb = ldvec(fb1d, FT, "fb1") if fl.fb1 else None
        fb2sb = ldvec(fb2d, DT, "fb2") if fl.fb2 else None
        lns = {k: (ldvec(v, DT, "ln" + k) if v is not None else None)
               for k, v in lnp.items()}
        vb1sb = None
        if fl.vb1:
            vb1sb = const.tile([P, D], f32, tag="vb1")
            nc.sync.dma_start(out=vb1sb[:, :], in_=vb1[:, :])
        vb2sb = None
        if fl.vb2:
            vb2sb = const.tile([P, D], f32, tag="vb2")
            nc.sync.dma_start(out=vb2sb[:, :], in_=vb2[:, :])
        kb2sb = None
        if fl.kb2:
            kb2sb = const.tile([P, NT], f32, tag="kb2")
            nc.sync.dma_start(out=kb2sb[:, :], in_=kb2.rearrange("n p o -> p (n o)"))

        # qT/aoT are assigned later (mid pool); closures below late-bind.
        qT = aoT = None

        # =========== QKV projection phase (local quarter + exchange) ===========
        def gather(loc, g_out):
            if cfg.fake_gather:
                n = loc.shape[0]
                for g in range(4):
                    nc.sync.dma_start(out=g_out[g * n : (g + 1) * n],
                                      in_=loc[:])
            else:
                nc.gpsimd.collective_compute(
                    "AllGather",
                    bass.mybir.AluOpType.bypass,
                    replica_groups=[[0, 1, 2, 3], [4, 5, 6, 7]],
                    ins=[loc[:]],
                    outs=[g_out[:]],
                )

        def qkv_phase(xqn, xq, wT, kTsb, vsb, qkb, vbsb, kloc, kg, vloc, vg):
            """K/V projected from the LOCAL token quarter xqn [P, DT, W] and
            all-gathered within the 4-core batch group; Q^T from xq
            (interleaved local queries). K gather overlaps V compute; V
            gather overlaps Q compute."""
            with tc.tile_pool(name="qkv_w", bufs=2) as wp, \
                 tc.tile_pool(name="qkv_wall", bufs=1) as wallp, \
                 tc.tile_pool(name="qkv_loc", bufs=1) as locp, \
                 tc.tile_pool(name="qkv_st", bufs=2) as stp, \
                 tc.tile_pool(name="qkv_ps", bufs=2, space="PSUM") as psp:
                # Prefetch V/Q weights up front: DMA issued during a gather
                # gets starved behind the collective's transfers.
                wvall = wallp.tile([P, DT, D], bf16, tag="wvall")
                nc.sync.dma_start(
                    out=wvall[:, :, :],
                    in_=wT[:, 2 * D : 3 * D].rearrange("(t p) v -> p t v", p=P),
                )
                wqall = wallp.tile([P, DT, D], bf16, tag="wqall")
                nc.sync.dma_start(
                    out=wqall[:, :, :],
                    in_=wT[:, 0:D].rearrange("(t p) v -> p t v", p=P),
                )
                # ---- K^T local [D, W] -> stage -> gather ----
                klocsb = locp.tile([P, DT, W], bf16, tag="kloc")
                for dk in range(DT):
                    wk = wp.tile([P, DT, P], bf16, tag="wk")
                    nc.sync.dma_start(
                        out=wk[:, :, :],
                        in_=wT[:, D + dk * P : D + (dk + 1) * P]
                        .rearrange("(t p) v -> p t v", p=P),
                    )
                    ps = psp.tile([P, W], f32, tag="kps")
                    for dt in range(DT):
                        nc.tensor.matmul(
                            ps[:, :],
                            lhsT=wk[:, dt, :],
                            rhs=xqn[:, dt, :],
                            start=(dt == 0),
                            stop=(dt == DT - 1),
                        )
                    if qkb is not None:
                        nc.scalar.activation(
                            out=klocsb[:, dk, :], in_=ps[:, :],
                            func=AF.Identity,
                            bias=qkb[:, DT + dk : DT + dk + 1], scale=1.0,
                        )
                    else:
                        nc.scalar.activation(
                            out=klocsb[:, dk, :], in_=ps[:, :], func=AF.Copy,
                        )
                nc.sync.dma_start(
                    out=kloc[:].rearrange("(t p w) -> p t w", p=P, w=W),
                    in_=klocsb[:, :, :],
                )
                gather(kloc, kg)
                # ---- V local [n, dout] + ones column -> stage -> gather ----
                vlocsb = locp.tile([P, NTQ, H, HD1], bf16, tag="vloc")
                nc.vector.memset(vlocsb[:, :, :, HD:HD1], 1.0)
                for vc in range(VCN):
                    for nt in range(NTQ):
                        ps = psp.tile([P, VCW], f32, tag="vps")
                        for dt in range(DT):
                            nc.tensor.matmul(
                                ps[:, :],
                                lhsT=xqn[:, dt, nt * P : (nt + 1) * P],
                                rhs=wvall[:, dt, vc * VCW : (vc + 1) * VCW],
                                start=(dt == 0),
                                stop=(dt == DT - 1),
                            )
                        vdst = vlocsb[:, nt, vc * HPC : (vc + 1) * HPC, 0:HD]
                        if vbsb is not None:
                            st = stp.tile([P, HPC, HD], f32, tag="vst")
                            nc.vector.tensor_add(
                                st[:, :, :],
                                ps.rearrange("p (h d) -> p h d", d=HD),
                                vbsb[:, vc * VCW : (vc + 1) * VCW].rearrange(
                                    "p (h d) -> p h d", d=HD),
                            )
                            nc.scalar.activation(
                                out=vdst, in_=st[:, :, :], func=AF.Copy)
                        else:
                            nc.scalar.activation(
                                out=vdst,
                                in_=ps.rearrange("p (h d) -> p h d", d=HD),
                                func=AF.Copy,
                            )
                nc.sync.dma_start(
                    out=vloc[:].rearrange("(n p x) -> p n x", p=P, x=HX),
                    in_=vlocsb.rearrange("p n h d -> p n (h d)"),
                )
                gather(vloc, vg)
                # ---- Q^T local [D, W] (overlaps the V gather) ----
                for dq in range(DT):
                    ps = psp.tile([P, W], f32, tag="qps")
                    for dt in range(DT):
                        nc.tensor.matmul(
                            ps[:, :],
                            lhsT=wqall[:, dt, dq * P : (dq + 1) * P],
                            rhs=xq[:, dt, :],
                            start=(dt == 0),
                            stop=(dt == DT - 1),
                        )
                    if qkb is not None:
                        nc.scalar.activation(
                            out=qT[:, dq, :], in_=ps[:, :], func=AF.Identity,
                            bias=qkb[:, dq : dq + 1], scale=1.0,
                        )
                    else:
                        nc.scalar.activation(
                            out=qT[:, dq, :], in_=ps[:, :], func=AF.Copy,
                        )
                # ---- land gathered K^T / V into SBUF (two DMA queues) ----
                for g in range(4):
                    nc.sync.dma_start(
                        out=kTsb[:, :, g * W : (g + 1) * W],
                        in_=kg[g * KL : (g + 1) * KL]
                        .rearrange("(t p w) -> p t w", p=P, w=W),
                    )
                    nc.scalar.dma_start(
                        out=vsb[:, g * NTQ : (g + 1) * NTQ, :, :]
                        .rearrange("p n h d -> p n (h d)"),
                        in_=vg[g * VL : (g + 1) * VL]
                        .rearrange("(n p x) -> p n x", p=P, x=HX),
                    )

        # =========== attention phase ===========
        def attn_phase(kTsb, vsb, m1sb, use_kb2, causal=False):
            """causal: queries are interleaved (local col j = global query
            4j+r), so key-tile group g is only needed by columns
            >= KTG*32*g — uniformly across cores. Skip the earlier columns."""
            def jg_of(g):
                return min(W, KTG * 32 * g) if causal else 0

            with tc.tile_pool(name="at_ex", bufs=2) as exp_, \
                 tc.tile_pool(name="at_dn", bufs=1) as dnp, \
                 tc.tile_pool(name="at_ps", bufs=2, space="PSUM") as psp, \
                 tc.tile_pool(name="at_po", bufs=2, space="PSUM") as pop, \
                 tc.tile_pool(name="at_pb", bufs=2, space="PSUM") as pbp:
                dall = dnp.tile([P, W], f32, tag="dall")
                for h in range(H):
                    hh = (h % HP) * HD  # partition base shared with q_h
                    dth = h // HP
                    q_h = qT[hh : hh + HD, dth, :]
                    po = pop.tile([P, W], f32, tag="po")
                    ng = sum(1 for g in range(NG) if jg_of(g) < W)
                    for g in range(ng):
                        jg = jg_of(g)
                        ps = psp.tile([P, KTG, W], f32, tag="sc")
                        for o in range(KTG):
                            kt = g * KTG + o
                            nc.tensor.matmul(
                                ps[:, o, jg:W],
                                lhsT=kTsb[hh : hh + HD, dth,
                                          kt * P : (kt + 1) * P],
                                rhs=q_h[:, jg:W],
                                start=True,
                                stop=True,
                            )
                        ex = exp_.tile([P, KTG, W], bf16, tag="ex")
                        if use_kb2:
                            for o in range(KTG):
                                kt = g * KTG + o
                                nc.scalar.activation(
                                    out=ex[:, o, jg:W], in_=ps[:, o, jg:W],
                                    func=AF.Exp,
                                    bias=kb2sb[:, kt : kt + 1],
                                    scale=1.0 / np.sqrt(HD),
                                )
                        else:
                            nc.scalar.activation(
                                out=ex[:, :, jg:W], in_=ps[:, :, jg:W],
                                func=AF.Exp,
                                scale=1.0 / np.sqrt(HD),
                            )
                        if m1sb is not None:
                            nc.vector.tensor_mul(
                                ex[:, :, jg:W], ex[:, :, jg:W],
                                m1sb[:, g * KTG : (g + 1) * KTG, jg:W],
                            )
                        for o in range(KTG):
                            kt = g * KTG + o
                            nc.tensor.matmul(
                                po[0:HD1, jg:W],
                                lhsT=vsb[:, kt, h, :],
                                rhs=ex[:, o, jg:W],
                                start=(g == 0 and o == 0),
                                stop=(g == ng - 1 and o == KTG - 1),
                            )
                    # unnormalized head output + denominator row out of PSUM
                    nc.scalar.activation(
                        out=aoT[hh : hh + HD, dth, :], in_=po[0:HD, :],
                        func=AF.Copy,
                    )
                    s64 = exp_.tile([HD1, W], f32, tag="s64")
                    nc.vector.tensor_copy(s64[HD:HD1, :], po[HD:HD1, :])
                    nc.sync.dma_start(out=dall[h : h + 1, :],
                                      in_=s64[HD:HD1, :])
                # batched softmax normalization
                nc.vector.reciprocal(dall[0:H, :], dall[0:H, :])
                for h in range(H):
                    hh = (h % HP) * HD
                    dth = h // HP
                    d1 = exp_.tile([1, W], f32, tag="d1")
                    nc.sync.dma_start(out=d1[0:1, :], in_=dall[h : h + 1, :])
                    pb = pbp.tile([HD, W], f32, tag="pb")
                    nc.tensor.matmul(
                        pb[0:HD, :],
                        lhsT=ones_hd[0:1, :],
                        rhs=d1[0:1, :],
                        start=True, stop=True,
                    )
                    nc.vector.tensor_mul(
                        aoT[hh : hh + HD, dth, :],
                        aoT[hh : hh + HD, dth, :],
                        pb[0:HD, :],
                    )

        # =========== layernorm (transposed layout) ===========
        def ln_t(pre, out_t, g_sb, b_sb, lpp, lp, lst):
            acc = lp.tile([P, W], f32, tag="lnacc")
            nc.vector.tensor_add(acc[:, :], pre[:, 0, :], pre[:, 1, :])
            for d in range(2, DT):
                nc.vector.tensor_add(acc[:, :], acc[:, :], pre[:, d, :])
            sqa = lp.tile([P, W], f32, tag="lnsqa")
            nc.scalar.square(sqa[:, :], pre[:, 0, :])
            for d in range(1, DT):
                sqt = lp.tile([P, W], f32, tag="lnsqt")
                nc.scalar.square(sqt[:, :], pre[:, d, :])
                nc.vector.tensor_add(sqa[:, :], sqa[:, :], sqt[:, :])
            sums = lpp.tile([1, W], f32, tag="lnsums")
            nc.tensor.matmul(sums[0:1, :], lhsT=ones_p1[:, :],
                             rhs=acc[:, :], start=True, stop=True)
            sqs = lpp.tile([1, W], f32, tag="lnsqs")
            nc.tensor.matmul(sqs[0:1, :], lhsT=ones_p1[:, :],
                             rhs=sqa[:, :], start=True, stop=True)
            mu = lst.tile([1, W], f32, tag="lnmu")
            nc.vector.tensor_scalar_mul(mu[0:1, :], sums[0:1, :], 1.0 / D)
            ex2 = lst.tile([1, W], f32, tag="lnex2")
            nc.vector.tensor_scalar_mul(ex2[0:1, :], sqs[0:1, :], 1.0 / D)
            mu2 = lst.tile([1, W], f32, tag="lnmu2")
            nc.scalar.square(mu2[0:1, :], mu[0:1, :])
            var = lst.tile([1, W], f32, tag="lnvar")
            nc.vector.tensor_sub(var[0:1, :], ex2[0:1, :], mu2[0:1, :])
            sd = lst.tile([1, W], f32, tag="lnsd")
            nc.scalar.activation(out=sd[0:1, :], in_=var[0:1, :], func=AF.Sqrt,
                                 bias=eps_t[0:1, :], scale=1.0)
            rstd = lst.tile([1, W], f32, tag="lnrstd")
            nc.vector.reciprocal(rstd[0:1, :], sd[0:1, :])
            mub = lpp.tile([P, W], f32, tag="lnmub")
            nc.tensor.matmul(mub[:, :], lhsT=ones_1p[0:1, :],
                             rhs=mu[0:1, :], start=True, stop=True)
            rstdb = lpp.tile([P, W], f32, tag="lnrstdb")
            nc.tensor.matmul(rstdb[:, :], lhsT=ones_1p[0:1, :],
                             rhs=rstd[0:1, :], start=True, stop=True)
            for d in range(DT):
                t1 = lp.tile([P, W], f32, tag="lnt1")
                nc.vector.tensor_sub(t1[:, :], pre[:, d, :], mub[:, :])
                nc.vector.tensor_mul(out_t[:, d, :], t1[:, :], rstdb[:, :])
                if g_sb is not None:
                    nc.vector.tensor_scalar_mul(
                        out_t[:, d, :], out_t[:, d, :], g_sb[:, d : d + 1])
                if b_sb is not None:
                    nc.vector.tensor_scalar_add(
                        out_t[:, d, :], out_t[:, d, :], b_sb[:, d : d + 1])

        # =========== out-projection + residual + LN ===========
        def proj_resid_ln(owT, obsb, residT, g_sb, b_sb, out_t, out_b):
            """out_t: fp32 LN output; out_b: bf16 copy (or None)."""
            with tc.tile_pool(name="pr_w", bufs=2) as wp, \
                 tc.tile_pool(name="pr_t", bufs=2) as lp, \
                 tc.tile_pool(name="pr_st", bufs=1) as lst, \
                 tc.tile_pool(name="pr_pre", bufs=1) as prep, \
                 tc.tile_pool(name="pr_ps", bufs=2, space="PSUM") as psp, \
                 tc.tile_pool(name="pr_lnps", bufs=1, space="PSUM") as lpp:
                pre = prep.tile([P, DT, W], f32, tag="pre")
                G4 = min(4, DT)
                for dg in range(DT // G4):
                    wsl = wp.tile([P, DT, G4 * P], bf16, tag="prw")
                    nc.sync.dma_start(
                        out=wsl[:, :, :],
                        in_=owT[:, dg * G4 * P : (dg + 1) * G4 * P]
                        .rearrange("(t p) v -> p t v", p=P),
                    )
                    for j in range(G4):
                        d = dg * G4 + j
                        ps = psp.tile([P, W], f32, tag="prps")
                        for dt in range(DT):
                            nc.tensor.matmul(
                                ps[:, :], lhsT=wsl[:, dt, j * P : (j + 1) * P],
                                rhs=aoT[:, dt, :],
                                start=(dt == 0), stop=(dt == DT - 1),
                            )
                        if obsb is not None:
                            tmp = lp.tile([P, W], f32, tag="prtmp")
                            nc.scalar.activation(out=tmp[:, :], in_=ps[:, :],
                                                 func=AF.Identity,
                                                 bias=obsb[:, d : d + 1], scale=1.0)
                            nc.vector.tensor_add(pre[:, d, :], tmp[:, :],
                                                 residT[:, d, :])
                        else:
                            nc.vector.tensor_add(pre[:, d, :], ps[:, :],
                                                 residT[:, d, :])
                ln_t(pre, out_t, g_sb, b_sb, lpp, lp, lst)
                if out_b is not None:
                    for d in range(DT):
                        nc.scalar.activation(out=out_b[:, d, :],
                                             in_=out_t[:, d, :], func=AF.Copy)

        # ================= pipeline =================
        midp = es.enter_context(tc.tile_pool(name="mid", bufs=1))
        qT = midp.tile([P, DT, W], bf16)     # Q^T local (reused block2)
        aoT = midp.tile([P, DT, W], bf16)    # attention out^T (reused)
        x1T = midp.tile([P, DT, W], f32)     # x1 local fp32 (residual 2)
        x1b = midp.tile([P, DT, W], bf16)    # x1 local bf16 (matmul source)

        with tc.tile_pool(name="kv1", bufs=1) as kvp1:
            kT1sb = kvp1.tile([P, DT, S], bf16)
            v1sb = kvp1.tile([P, NT, H, HD1], bf16)
            with tc.tile_pool(name="xt1", bufs=1) as xtp1:
                xqn1 = xtp1.tile([P, DT, W], bf16)
                nc.sync.dma_start(out=xqn1[:, :, :],
                                  in_=xTqb.rearrange("(t p) s -> p t s", p=P))
                xlb1 = xtp1.tile([P, DT, W], bf16)
                nc.sync.dma_start(out=xlb1[:, :, :],
                                  in_=xTlb.rearrange("(t p) s -> p t s", p=P))
                qkv_phase(xqn1, xlb1, qkvwT1, kT1sb, v1sb, qkb1sb, vb1sb,
                          kloc1, kg1, vloc1, vg1)

            with tc.tile_pool(name="xtl", bufs=1) as xtlp:
                xTlt = xtlp.tile([P, DT, W], f32)
                nc.sync.dma_start(out=xTlt[:, :, :],
                                  in_=xTl.rearrange("(t p) s -> p t s", p=P))
                if fl.m1:
                    with tc.tile_pool(name="m1p", bufs=1) as m1p:
                        m1sb = m1p.tile([P, NT, W], bf16)
                        nc.sync.dma_start(out=m1sb[:, :, :],
                                          in_=m1.rearrange("n p w -> p n w"))
                        attn_phase(kT1sb, v1sb, m1sb, False,
                                   causal=fl.causal)
                else:
                    attn_phase(kT1sb, v1sb, None, False)

                proj_resid_ln(owT1, ob1sb, xTlt, lns["g1"], lns["b1"],
                              x1T, x1b)

        x2p = es.enter_context(tc.tile_pool(name="x2p", bufs=1))
        x2T = x2p.tile([P, DT, W], f32)
        x2b = x2p.tile([P, DT, W], bf16)

        with tc.tile_pool(name="kv2", bufs=1) as kvp2:
            kT2sb = kvp2.tile([P, DT, S], bf16)
            v2sb = kvp2.tile([P, NT, H, HD1], bf16)
            # block-2 K/V from the core's own (interleaved) x1 columns;
            # gathered key n = g*W + w is token 4w+g, fine for unmasked
            # cross-attention (kb2 data is host-permuted to match).
            qkv_phase(x1b, x1b, qkvwT2, kT2sb, v2sb, qkb2sb, vb2sb,
                      kloc2, kg2, vloc2, vg2)

            attn_phase(kT2sb, v2sb, None, fl.kb2)

            proj_resid_ln(owT2, ob2sb, x1T, lns["g2"], lns["b2"], x2T, x2b)

        # ================= FFN =================
        with tc.tile_pool(name="ffh", bufs=1) as fhp, \
             tc.tile_pool(name="ffw", bufs=2) as wp, \
             tc.tile_pool(name="fft", bufs=2) as lp, \
             tc.tile_pool(name="ffst", bufs=1) as lst, \
             tc.tile_pool(name="ffpre", bufs=1) as prep:
            hT = fhp.tile([P, FT, W], bf16)
            G4 = min(4, DT)
            with tc.tile_pool(name="ffps1", bufs=2, space="PSUM") as psp:
                for fg in range(FT // G4):
                    wsl = wp.tile([P, DT, G4 * P], bf16, tag="f1w")
                    nc.sync.dma_start(
                        out=wsl[:, :, :],
                        in_=w1T[:, fg * G4 * P : (fg + 1) * G4 * P]
                        .rearrange("(t p) v -> p t v", p=P),
                    )
                    for j in range(G4):
                        f = fg * G4 + j
                        ps = psp.tile([P, W], f32, tag="f1ps")
                        for dt in range(DT):
                            nc.tensor.matmul(
                                ps[:, :], lhsT=wsl[:, dt, j * P : (j + 1) * P],
                                rhs=x2b[:, dt, :],
                                start=(dt == 0), stop=(dt == DT - 1),
                            )
                        if fb1sb is not None:
                            nc.scalar.activation(out=hT[:, f, :], in_=ps[:, :],
                                                 func=AF.Relu,
                                                 bias=fb1sb[:, f : f + 1], scale=1.0)
                        else:
                            nc.scalar.activation(out=hT[:, f, :], in_=ps[:, :],
                                                 func=AF.Relu)
            pre = prep.tile([P, DT, W], f32, tag="ffpre")
            with tc.tile_pool(name="ffps2", bufs=1, space="PSUM") as psq, \
                 tc.tile_pool(name="fflnps", bufs=1, space="PSUM") as lpp:
                for dg in range(DT // G4):
                    ps4 = []
                    for j in range(G4):
                        ps4j = psq.tile([P, W], f32, tag="f2ps%d" % j)
                        ps4.append(ps4j)
                    for ft in range(FT):
                        wsl = wp.tile([P, G4 * P], bf16, tag="f2w")
                        nc.sync.dma_start(
                            out=wsl[:, :],
                            in_=w2T[ft * P : (ft + 1) * P,
                                    dg * G4 * P : (dg + 1) * G4 * P],
                        )
                        for j in range(G4):
                            nc.tensor.matmul(
                                ps4[j][:, :],
                                lhsT=wsl[:, j * P : (j + 1) * P],
                                rhs=hT[:, ft, :],
                                start=(ft == 0), stop=(ft == FT - 1),
                            )
                    for j in range(G4):
                        d = dg * G4 + j
                        if fb2sb is not None:
                            tmp = lp.tile([P, W], f32, tag="f2tmp")
                            nc.scalar.activation(out=tmp[:, :], in_=ps4[j][:, :],
                                                 func=AF.Identity,
                                                 bias=fb2sb[:, d : d + 1], scale=1.0)
                            nc.vector.tensor_add(pre[:, d, :], tmp[:, :],
                                                 x2T[:, d, :])
                        else:
                            nc.vector.tensor_add(pre[:, d, :], ps4[j][:, :],
                                                 x2T[:, d, :])
                ln_t(pre, pre, lns["g3"], lns["b3"], lpp, lp, lst)
                for d in range(DT):
                    nc.sync.dma_start(out=out[d * P : (d + 1) * P, :],
                                      in_=pre[:, d, :])


def make_program(cfg, fl):
    from concourse import bacc
    import concourse.tile as tile

    nc = bacc.Bacc("TRN2", target_bir_lowering=False, debug=False,
                   num_devices=8)
    with tile.TileContext(nc) as tc:
        _build(nc, tc, cfg, fl)
    nc.compile()
    return nc


def prep_inputs(inputs, cfg):
    """Host-side data prep. Returns (in_maps, fl)."""
    import ml_dtypes

    B, S, D, H, DFF, W, NT = (cfg.B, cfg.S, cfg.D, cfg.H, cfg.DFF,
                              cfg.W, cfg.NT)
    f = np.float32
    bf = ml_dtypes.bfloat16
    x = np.asarray(inputs["x"], f)
    enc = np.asarray(inputs["enc_out"])
    trg = np.asarray(inputs["trg_mask"])
    fl = Flags()
    fl.qkb1 = bool(np.any(inputs["qkv_b1"]))
    fl.qkb2 = bool(np.any(inputs["qkv_b2"]))
    fl.vb1 = bool(np.any(np.asarray(inputs["qkv_b1"])[2 * D :]))
    fl.vb2 = bool(np.any(np.asarray(inputs["qkv_b2"])[2 * D :]))
    fl.ob1 = bool(np.any(inputs["out_b1"]))
    fl.ob2 = bool(np.any(inputs["out_b2"]))
    fl.fb1 = bool(np.any(inputs["ff_b1"]))
    fl.fb2 = bool(np.any(inputs["ff_b2"]))
    fl.g1 = not bool(np.all(np.asarray(inputs["ln1_g"]) == 1))
    fl.b1 = bool(np.any(inputs["ln1_b"]))
    fl.g2 = not bool(np.all(np.asarray(inputs["ln2_g"]) == 1))
    fl.b2 = bool(np.any(inputs["ln2_b"]))
    fl.g3 = not bool(np.all(np.asarray(inputs["ln3_g"]) == 1))
    fl.b3 = bool(np.any(inputs["ln3_b"]))
    fl.m1 = not bool(np.all(trg != 0))
    fl.kb2 = bool(np.any(enc == 0))
    # causal <=> no mask entries above the diagonal (so key > query can be
    # skipped statically); queries are interleaved (core r gets q = r::4)
    # which makes the per-column needed-key count uniform across cores.
    if fl.m1:
        # trg[b, 0, q, k]: entries with k > q are the strictly-upper triangle
        fl.causal = not bool(np.any(np.triu(trg[:, 0], 1)))

    shared = {
        "qkvwT1": np.ascontiguousarray(np.asarray(inputs["qkv_w1"], f).T).astype(bf),
        "qkvwT2": np.ascontiguousarray(np.asarray(inputs["qkv_w2"], f).T).astype(bf),
        "owT1": np.ascontiguousarray(np.asarray(inputs["out_w1"], f).T).astype(bf),
        "owT2": np.ascontiguousarray(np.asarray(inputs["out_w2"], f).T).astype(bf),
        "w1T": np.ascontiguousarray(np.asarray(inputs["ff_w1"], f).T).astype(bf),
        "w2T": np.ascontiguousarray(np.asarray(inputs["ff_w2"], f).T).astype(bf),
    }
    if fl.qkb1:
        shared["qkvb1"] = np.asarray(inputs["qkv_b1"], f)
    if fl.qkb2:
        shared["qkvb2"] = np.asarray(inputs["qkv_b2"], f)
    if fl.vb1:
        shared["vb1"] = np.broadcast_to(
            np.asarray(inputs["qkv_b1"], f)[2 * D :], (P, D)).copy()
    if fl.vb2:
        shared["vb2"] = np.broadcast_to(
            np.asarray(inputs["qkv_b2"], f)[2 * D :], (P, D)).copy()
    if fl.ob1:
        shared["ob1"] = np.asarray(inputs["out_b1"], f)
    if fl.ob2:
        shared["ob2"] = np.asarray(inputs["out_b2"], f)
    if fl.fb1:
        shared["fb1"] = np.asarray(inputs["ff_b1"], f)
    if fl.fb2:
        shared["fb2"] = np.asarray(inputs["ff_b2"], f)
    for nm, key, use in [("g1", "ln1_g", fl.g1), ("b1", "ln1_b", fl.b1),
                         ("g2", "ln2_g", fl.g2), ("b2", "ln2_b", fl.b2),
                         ("g3", "ln3_g", fl.g3), ("b3", "ln3_b", fl.b3)]:
        if use:
            shared[nm] = np.asarray(inputs[key], f)

    xTb_ = [np.ascontiguousarray(x[b].T) for b in range(B)]
    xTbf = [t.astype(bf) for t in xTb_]
    # token held at gathered sequence position n = g*W + w is 4*w + g
    # (core g's local column w is global query 4w+g)
    tok_of_n = 4 * (np.arange(S) % W) + (np.arange(S) // W)
    in_maps = []
    for c in range(8):
        b, r = c // 4, c % 4
        qidx = np.arange(r, S, 4)  # this core's (interleaved) queries
        m = dict(shared)
        m["xTqb"] = np.ascontiguousarray(xTbf[b][:, r * W : (r + 1) * W])
        m["xTlb"] = np.ascontiguousarray(xTbf[b][:, qidx])
        m["xTl"] = np.ascontiguousarray(xTb_[b][:, qidx])
        if fl.m1:
            # m1[kt, i, j] = trg[0or b, 0, qidx[j], kt*P + i]  (0/1)
            tb = trg[b] if trg.shape[0] == B else trg[0]
            blk = tb[0, qidx, :]  # [W, S] (q, k)
            m["m1"] = np.ascontiguousarray(
                (blk.T != 0).astype(bf).reshape(NT, P, W))
        if fl.kb2:
            eb = enc[b, 0, 0, :]  # [S], indexed by token
            kbv = np.where(eb[tok_of_n] != 0, f(0.0), f(-1e20)).astype(f)
            m["kb2"] = kbv.reshape(NT, P, 1)
        in_maps.append(m)
    return in_maps, fl


def kernel_with_results(_run_kwargs=None, **inputs):
    from concourse.bass_utils import run_bass_kernel_spmd

    cfg = Cfg()
    x = np.asarray(inputs["x"])
    assert x.shape == (cfg.B, cfg.S, cfg.D), x.shape
    in_maps, fl = prep_inputs(inputs, cfg)
    nc = make_program(cfg, fl)
    res = run_bass_kernel_spmd(nc, in_maps, list(range(8)),
                               **(_run_kwargs or {}))
    y = np.empty((cfg.B, cfg.S, cfg.D), np.float32)
    for c in range(8):
        b, r = c // 4, c % 4
        y[b, r::4, :] = res.results[c]["out"].T
    return y, res


def kernel(**inputs):
    return kernel_with_results(**inputs)[0]


# revision 42
# speedup vs baseline: 1.0011x; 1.0011x over previous
"""Trainium2 Bass kernel for nn_DecoderLayer (dense transformer decoder layer).

Strategy (8 NeuronCores, full inputs in / full output out):
  - core c handles batch b = c//4 and query-quarter r = c%4 (rows [r*S/4, (r+1)*S/4)).
  - All matmul operands are bf16 (weights cast host-side, activations cast at
    the PSUM->SBUF copy points); PSUM accumulation stays fp32, as do residuals
    and LayerNorm statistics.
  - K^T and V(+ones column) live entirely in SBUF (no DRAM round-trip); x^T is
    loaded to SBUF once per block and sliced for the K/V/Q projections.
  - Attention per head: S^T[k, q] scores via PE (64-partition contraction),
    exp on ACT (1/8 scale fused, bf16 out), causal mask as multiplicative
    bf16 [128, W] tiles, softmax denominators free via a ones column in V,
    normalization deferred: unnormalized head outputs + per-head denominator
    rows are copied out of PSUM, one batched reciprocal [H, W] per block, then
    per-head PE ones-broadcast + vector multiply.
  - The single collective: bf16 AllGather of x1 (post-LN1) within each 4-core
    batch group; the Q2 projection is issued after it so it overlaps.
  - LayerNorm in transposed layout: cross-partition sums via ones-matmul on
    the PE, stats broadcast back to [128, W] via ones-matmul.
"""

import sys

if "/opt/trn_rl_repo" not in sys.path:
    sys.path.insert(0, "/opt/trn_rl_repo")

import numpy as np

P = 128
HD = 64
HD1 = HD + 1
EPS = 1e-5


class Cfg:
    def __init__(self, B=2, S=2048, D=1024, H=16, DFF=4096, use_collective=True,
                 fake_gather=False):
        self.B, self.S, self.D, self.H, self.DFF = B, S, D, H, DFF
        self.fake_gather = fake_gather
        self.W = S // 4            # local query rows per core
        self.DT = D // P           # feature-dim tiles
        self.NT = S // P           # sequence tiles (keys)
        self.FT = DFF // P         # ffn hidden tiles
        self.HP = P // HD          # heads per partition-tile (2)
        self.NCH = max(1, S // 512)   # n-chunks for K-orientation matmuls
        self.NCW = S // self.NCH      # n-chunk width (<=512)
        self.VCW = min(512, D)        # v-dout chunk width
        self.VCN = D // self.VCW
        self.KTG = 2                  # k-tiles per exp group
        self.NG = self.NT // self.KTG
        self.use_collective = use_collective
        assert D == H * HD
        assert self.W % P == 0 and D % P == 0 and DFF % P == 0 and S % P == 0
        assert self.NT % self.KTG == 0


class Flags:
    def __init__(self):
        self.qkb1 = self.vb1 = self.ob1 = False
        self.qkb2 = self.vb2 = self.ob2 = False
        self.fb1 = self.fb2 = False
        self.g1 = self.b1 = self.g2 = self.b2 = self.g3 = self.b3 = False
        self.m1 = True      # trg mask multiplicative tiles
        self.kb2 = False    # enc mask additive per-k bias
        self.causal = False  # trg mask is lower-triangular -> column skip


def _build(nc, tc, cfg, fl):
    import concourse.bass as bass
    import concourse.mybir as mybir
    import concourse.tile as tile  # noqa: F401
    from contextlib import ExitStack

    AF = mybir.ActivationFunctionType
    f32 = mybir.dt.float32
    bf16 = mybir.dt.bfloat16

    B, S, D, H, DFF = cfg.B, cfg.S, cfg.D, cfg.H, cfg.DFF
    W, DT, NT, FT, HP = cfg.W, cfg.DT, cfg.NT, cfg.FT, cfg.HP
    NCH, NCW, VCW, VCN = cfg.NCH, cfg.NCW, cfg.VCW, cfg.VCN
    KTG, NG = cfg.KTG, cfg.NG
    HPC = VCW // HD  # heads per v-chunk

    # ---------------- DRAM parameters ----------------
    def din(name, shape, dt=f32):
        return nc.dram_tensor(name, shape, dt, kind="ExternalInput").ap()

    xTb = din("xTb", [D, S], bf16)       # full x^T (block-1 K/V source)
    xTlb = din("xTlb", [D, W], bf16)     # interleaved queries, bf16 (Q source)
    xTl = din("xTl", [D, W])             # interleaved queries, fp32 (residual)
    qkvwT1 = din("qkvwT1", [D, 3 * D], bf16)
    qkvwT2 = din("qkvwT2", [D, 3 * D], bf16)
    owT1 = din("owT1", [D, D], bf16)
    owT2 = din("owT2", [D, D], bf16)
    w1T = din("w1T", [D, DFF], bf16)
    w2T = din("w2T", [DFF, D], bf16)
    m1 = din("m1", [NT, P, W], bf16) if fl.m1 else None
    kb2 = din("kb2", [NT, P, 1]) if fl.kb2 else None
    qkvb1 = din("qkvb1", [3 * D]) if fl.qkb1 else None
    qkvb2 = din("qkvb2", [3 * D]) if fl.qkb2 else None
    vb1 = din("vb1", [P, D]) if fl.vb1 else None
    vb2 = din("vb2", [P, D]) if fl.vb2 else None
    ob1 = din("ob1", [D]) if fl.ob1 else None
    ob2 = din("ob2", [D]) if fl.ob2 else None
    fb1d = din("fb1", [DFF]) if fl.fb1 else None
    fb2d = din("fb2", [D]) if fl.fb2 else None
    lnp = {}
    for nm, use in [("g1", fl.g1), ("b1", fl.b1), ("g2", fl.g2),
                    ("b2", fl.b2), ("g3", fl.g3), ("b3", fl.b3)]:
        lnp[nm] = din(nm, [D]) if use else None
    out = nc.dram_tensor("out", [D, W], f32, kind="ExternalOutput").ap()

    NTQ = W // P          # token tiles in the local quarter
    KL = D * W            # K staging elements
    VL = NTQ * P * H * HD1  # V staging elements
    HX = H * HD1

    es = ExitStack()
    with es:
        dramp = es.enter_context(tc.tile_pool(name="dram", bufs=1, space="DRAM"))
        kloc2 = dramp.tile([KL], bf16)
        kg2 = dramp.tile([4 * KL], bf16)
        vloc2 = dramp.tile([VL], bf16)
        vg2 = dramp.tile([4 * VL], bf16)

        const = es.enter_context(tc.tile_pool(name="const", bufs=1))
        ones_p1 = const.tile([P, 1], f32)
        nc.vector.memset(ones_p1[:, :], 1.0)
        ones_1p = const.tile([1, P], f32)
        nc.vector.memset(ones_1p[0:1, :], 1.0)
        ones_hd = const.tile([P, HD], f32)
        nc.vector.memset(ones_hd[:, :], 1.0)
        eps_t = const.tile([1, 1], f32)
        nc.vector.memset(eps_t[0:1, :], EPS)

        def ldvec(dram_vec, n_tiles, name):
            """[D]-style vector -> [P, n_tiles] sbuf tile (per-partition slices)."""
            t = const.tile([P, n_tiles], f32, tag=name)
            nc.sync.dma_start(
                out=t[:, :],
                in_=dram_vec.rearrange("(t p) -> p t", p=P),
            )
            return t

        qkb1sb = ldvec(qkvb1[0 : 2 * D], 2 * DT, "qkb1") if fl.qkb1 else None
        qkb2sb = ldvec(qkvb2[0 : 2 * D], 2 * DT, "qkb2") if fl.qkb2 else None
        ob1sb = ldvec(ob1, DT, "ob1") if fl.ob1 else None
        ob2sb = ldvec(ob2, DT, "ob2") if fl.ob2 else None
        fb1sb = ldvec(fb1d, FT, "fb1") if fl.fb1 else None
        fb2sb = ldvec(fb2d, DT, "fb2") if fl.fb2 else None
        lns = {k: (ldvec(v, DT, "ln" + k) if v is not None else None)
               for k, v in lnp.items()}
        vb1sb = None
        if fl.vb1:
            vb1sb = const.tile([P, D], f32, tag="vb1")
            nc.sync.dma_start(out=vb1sb[:, :], in_=vb1[:, :])
        vb2sb = None
        if fl.vb2:
            vb2sb = const.tile([P, D], f32, tag="vb2")
            nc.sync.dma_start(out=vb2sb[:, :], in_=vb2[:, :])
        kb2sb = None
        if fl.kb2:
            kb2sb = const.tile([P, NT], f32, tag="kb2")
            nc.sync.dma_start(out=kb2sb[:, :], in_=kb2.rearrange("n p o -> p (n o)"))

        # qT/aoT are assigned later (mid pool); closures below late-bind.
        qT = aoT = None

        # =========== QKV projection phase (local quarter + exchange) ===========
        def gather(loc, g_out):
            if cfg.fake_gather:
                n = loc.shape[0]
                for g in range(4):
                    nc.sync.dma_start(out=g_out[g * n : (g + 1) * n],
                                      in_=loc[:])
            else:
                nc.gpsimd.collective_compute(
                    "AllGather",
                    bass.mybir.AluOpType.bypass,
                    replica_groups=[[0, 1, 2, 3], [4, 5, 6, 7]],
                    ins=[loc[:]],
                    outs=[g_out[:]],
                )

        def qkv_full(xbf, xq, wT, kTsb, vsb, qkb, vbsb):
            """Block-1 path: K^T/V for the FULL sequence from resident x^T
            (redundant across the 4-core group, but keeps the PE busy with
            no collective on the critical path)."""
            with tc.tile_pool(name="qkf_w", bufs=2) as wp, \
                 tc.tile_pool(name="qkf_st", bufs=2) as stp, \
                 tc.tile_pool(name="qkf_ps", bufs=2, space="PSUM") as psp:
                for dk in range(DT):
                    wk = wp.tile([P, DT, P], bf16, tag="wk")
                    nc.sync.dma_start(
                        out=wk[:, :, :],
                        in_=wT[:, D + dk * P : D + (dk + 1) * P]
                        .rearrange("(t p) v -> p t v", p=P),
                    )
                    for nch in range(NCH):
                        ps = psp.tile([P, NCW], f32, tag="kps")
                        for dt in range(DT):
                            nc.tensor.matmul(
                                ps[:, :],
                                lhsT=wk[:, dt, :],
                                rhs=xbf[:, dt, nch * NCW : (nch + 1) * NCW],
                                start=(dt == 0),
                                stop=(dt == DT - 1),
                            )
                        if qkb is not None:
                            nc.scalar.activation(
                                out=kTsb[:, dk, nch * NCW : (nch + 1) * NCW],
                                in_=ps[:, :], func=AF.Identity,
                                bias=qkb[:, DT + dk : DT + dk + 1], scale=1.0,
                            )
                        else:
                            nc.scalar.activation(
                                out=kTsb[:, dk, nch * NCW : (nch + 1) * NCW],
                                in_=ps[:, :], func=AF.Copy,
                            )
                nc.vector.memset(vsb[:, :, :, HD:HD1], 1.0)
                for vc in range(VCN):
                    wv = wp.tile([P, DT, VCW], bf16, tag="wv")
                    nc.sync.dma_start(
                        out=wv[:, :, :],
                        in_=wT[:, 2 * D + vc * VCW : 2 * D + (vc + 1) * VCW]
                        .rearrange("(t p) v -> p t v", p=P),
                    )
                    for nt in range(NT):
                        ps = psp.tile([P, VCW], f32, tag="vps")
                        for dt in range(DT):
                            nc.tensor.matmul(
                                ps[:, :],
                                lhsT=xbf[:, dt, nt * P : (nt + 1) * P],
                                rhs=wv[:, dt, :],
                                start=(dt == 0),
                                stop=(dt == DT - 1),
                            )
                        vdst = vsb[:, nt, vc * HPC : (vc + 1) * HPC, 0:HD]
                        if vbsb is not None:
                            st = stp.tile([P, HPC, HD], f32, tag="vst")
                            nc.vector.tensor_add(
                                st[:, :, :],
                                ps.rearrange("p (h d) -> p h d", d=HD),
                                vbsb[:, vc * VCW : (vc + 1) * VCW].rearrange(
                                    "p (h d) -> p h d", d=HD),
                            )
                            nc.scalar.activation(
                                out=vdst, in_=st[:, :, :], func=AF.Copy)
                        else:
                            nc.scalar.activation(
                                out=vdst,
                                in_=ps.rearrange("p (h d) -> p h d", d=HD),
                                func=AF.Copy,
                            )
                for dq in range(DT):
                    wq = wp.tile([P, DT, P], bf16, tag="wq")
                    nc.sync.dma_start(
                        out=wq[:, :, :],
                        in_=wT[:, dq * P : (dq + 1) * P]
                        .rearrange("(t p) v -> p t v", p=P),
                    )
                    ps = psp.tile([P, W], f32, tag="qps")
                    for dt in range(DT):
                        nc.tensor.matmul(
                            ps[:, :],
                            lhsT=wq[:, dt, :],
                            rhs=xq[:, dt, :],
                            start=(dt == 0),
                            stop=(dt == DT - 1),
                        )
                    if qkb is not None:
                        nc.scalar.activation(
                            out=qT[:, dq, :], in_=ps[:, :], func=AF.Identity,
                            bias=qkb[:, dq : dq + 1], scale=1.0,
                        )
                    else:
                        nc.scalar.activation(
                            out=qT[:, dq, :], in_=ps[:, :], func=AF.Copy,
                        )

        def qkv_phase(xqn, xq, wT, kTsb, vsb, qkb, vbsb, kloc, kg, vloc, vg):
            """K/V projected from the LOCAL token quarter xqn [P, DT, W] and
            all-gathered within the 4-core batch group; Q^T from xq
            (interleaved local queries). K gather overlaps V compute; V
            gather overlaps Q compute."""
            with tc.tile_pool(name="qkv_w", bufs=2) as wp, \
                 tc.tile_pool(name="qkv_wall", bufs=1) as wallp, \
                 tc.tile_pool(name="qkv_loc", bufs=1) as locp, \
                 tc.tile_pool(name="qkv_st", bufs=2) as stp, \
                 tc.tile_pool(name="qkv_ps", bufs=2, space="PSUM") as psp:
                # Prefetch V/Q weights up front: DMA issued during a gather
                # gets starved behind the collective's transfers.
                wvall = wallp.tile([P, DT, D], bf16, tag="wvall")
                nc.sync.dma_start(
                    out=wvall[:, :, :],
                    in_=wT[:, 2 * D : 3 * D].rearrange("(t p) v -> p t v", p=P),
                )
                wqall = wallp.tile([P, DT, D], bf16, tag="wqall")
                nc.sync.dma_start(
                    out=wqall[:, :, :],
                    in_=wT[:, 0:D].rearrange("(t p) v -> p t v", p=P),
                )
                # ---- K^T local [D, W] -> stage -> gather ----
                klocsb = locp.tile([P, DT, W], bf16, tag="kloc")
                for dk in range(DT):
                    wk = wp.tile([P, DT, P], bf16, tag="wk")
                    nc.sync.dma_start(
                        out=wk[:, :, :],
                        in_=wT[:, D + dk * P : D + (dk + 1) * P]
                        .rearrange("(t p) v -> p t v", p=P),
                    )
                    ps = psp.tile([P, W], f32, tag="kps")
                    for dt in range(DT):
                        nc.tensor.matmul(
                            ps[:, :],
                            lhsT=wk[:, dt, :],
                            rhs=xqn[:, dt, :],
                            start=(dt == 0),
                            stop=(dt == DT - 1),
                        )
                    if qkb is not None:
                        nc.scalar.activation(
                            out=klocsb[:, dk, :], in_=ps[:, :],
                            func=AF.Identity,
                            bias=qkb[:, DT + dk : DT + dk + 1], scale=1.0,
                        )
                    else:
                        nc.scalar.activation(
                            out=klocsb[:, dk, :], in_=ps[:, :], func=AF.Copy,
                        )
                nc.sync.dma_start(
                    out=kloc[:].rearrange("(t p w) -> p t w", p=P, w=W),
                    in_=klocsb[:, :, :],
                )
                gather(kloc, kg)
                # ---- V local [n, dout] + ones column -> stage -> gather ----
                vlocsb = locp.tile([P, NTQ, H, HD1], bf16, tag="vloc")
                nc.vector.memset(vlocsb[:, :, :, HD:HD1], 1.0)
                for vc in range(VCN):
                    for nt in range(NTQ):
                        ps = psp.tile([P, VCW], f32, tag="vps")
                        for dt in range(DT):
                            nc.tensor.matmul(
                                ps[:, :],
                                lhsT=xqn[:, dt, nt * P : (nt + 1) * P],
                                rhs=wvall[:, dt, vc * VCW : (vc + 1) * VCW],
                                start=(dt == 0),
                                stop=(dt == DT - 1),
                            )
                        vdst = vlocsb[:, nt, vc * HPC : (vc + 1) * HPC, 0:HD]
                        if vbsb is not None:
                            st = stp.tile([P, HPC, HD], f32, tag="vst")
                            nc.vector.tensor_add(
                                st[:, :, :],
                                ps.rearrange("p (h d) -> p h d", d=HD),
                                vbsb[:, vc * VCW : (vc + 1) * VCW].rearrange(
                                    "p (h d) -> p h d", d=HD),
                            )
                            nc.scalar.activation(
                                out=vdst, in_=st[:, :, :], func=AF.Copy)
                        else:
                            nc.scalar.activation(
                                out=vdst,
                                in_=ps.rearrange("p (h d) -> p h d", d=HD),
                                func=AF.Copy,
                            )
                nc.sync.dma_start(
                    out=vloc[:].rearrange("(n p x) -> p n x", p=P, x=HX),
                    in_=vlocsb.rearrange("p n h d -> p n (h d)"),
                )
                gather(vloc, vg)
                # ---- Q^T local [D, W] (overlaps the V gather) ----
                for dq in range(DT):
                    ps = psp.tile([P, W], f32, tag="qps")
                    for dt in range(DT):
                        nc.tensor.matmul(
                            ps[:, :],
                            lhsT=wqall[:, dt, dq * P : (dq + 1) * P],
                            rhs=xq[:, dt, :],
                            start=(dt == 0),
                            stop=(dt == DT - 1),
                        )
                    if qkb is not None:
                        nc.scalar.activation(
                            out=qT[:, dq, :], in_=ps[:, :], func=AF.Identity,
                            bias=qkb[:, dq : dq + 1], scale=1.0,
                        )
                    else:
                        nc.scalar.activation(
                            out=qT[:, dq, :], in_=ps[:, :], func=AF.Copy,
                        )
                # ---- land gathered K^T / V into SBUF (two DMA queues) ----
                for g in range(4):
                    nc.sync.dma_start(
                        out=kTsb[:, :, g * W : (g + 1) * W],
                        in_=kg[g * KL : (g + 1) * KL]
                        .rearrange("(t p w) -> p t w", p=P, w=W),
                    )
                    eng = nc.sync if g % 2 == 0 else nc.scalar
                    eng.dma_start(
                        out=vsb[:, g * NTQ : (g + 1) * NTQ, :, :]
                        .rearrange("p n h d -> p n (h d)"),
                        in_=vg[g * VL : (g + 1) * VL]
                        .rearrange("(n p x) -> p n x", p=P, x=HX),
                    )

        # =========== attention phase ===========
        def attn_phase(kTsb, vsb, m1sb, use_kb2, causal=False):
            """causal: queries are interleaved (local col j = global query
            4j+r), so key-tile group g is only needed by columns
            >= KTG*32*g — uniformly across cores. Skip the earlier columns."""
            def jg_of(g):
                return min(W, KTG * 32 * g) if causal else 0

            with tc.tile_pool(name="at_ex", bufs=2) as exp_, \
                 tc.tile_pool(name="at_dn", bufs=1) as dnp, \
                 tc.tile_pool(name="at_ps", bufs=2, space="PSUM") as psp, \
                 tc.tile_pool(name="at_po", bufs=2, space="PSUM") as pop, \
                 tc.tile_pool(name="at_pb", bufs=2, space="PSUM") as pbp:
                dall = dnp.tile([P, W], f32, tag="dall")
                for h in range(H):
                    hh = (h % HP) * HD  # partition base shared with q_h
                    dth = h // HP
                    q_h = qT[hh : hh + HD, dth, :]
                    po = pop.tile([P, W], f32, tag="po")
                    ng = sum(1 for g in range(NG) if jg_of(g) < W)
                    for g in range(ng):
                        jg = jg_of(g)
                        ps = psp.tile([P, KTG, W], f32, tag="sc")
                        for o in range(KTG):
                            kt = g * KTG + o
                            nc.tensor.matmul(
                                ps[:, o, jg:W],
                                lhsT=kTsb[hh : hh + HD, dth,
                                          kt * P : (kt + 1) * P],
                                rhs=q_h[:, jg:W],
                                start=True,
                                stop=True,
                            )
                        ex = exp_.tile([P, KTG, W], bf16, tag="ex")
                        if use_kb2:
                            for o in range(KTG):
                                kt = g * KTG + o
                                nc.scalar.activation(
                                    out=ex[:, o, jg:W], in_=ps[:, o, jg:W],
                                    func=AF.Exp,
                                    bias=kb2sb[:, kt : kt + 1],
                                    scale=1.0 / np.sqrt(HD),
                                )
                        else:
                            nc.scalar.activation(
                                out=ex[:, :, jg:W], in_=ps[:, :, jg:W],
                                func=AF.Exp,
                                scale=1.0 / np.sqrt(HD),
                            )
                        if m1sb is not None:
                            nc.vector.tensor_mul(
                                ex[:, :, jg:W], ex[:, :, jg:W],
                                m1sb[:, g * KTG : (g + 1) * KTG, jg:W],
                            )
                        for o in range(KTG):
                            kt = g * KTG + o
                            nc.tensor.matmul(
                                po[0:HD1, jg:W],
                                lhsT=vsb[:, kt, h, :],
                                rhs=ex[:, o, jg:W],
                                start=(g == 0 and o == 0),
                                stop=(g == ng - 1 and o == KTG - 1),
                            )
                    # unnormalized head output + denominator row out of PSUM
                    nc.scalar.activation(
                        out=aoT[hh : hh + HD, dth, :], in_=po[0:HD, :],
                        func=AF.Copy,
                    )
                    s64 = exp_.tile([HD1, W], f32, tag="s64")
                    nc.vector.tensor_copy(s64[HD:HD1, :], po[HD:HD1, :])
                    nc.sync.dma_start(out=dall[h : h + 1, :],
                                      in_=s64[HD:HD1, :])
                # batched softmax normalization
                nc.vector.reciprocal(dall[0:H, :], dall[0:H, :])
                for h in range(H):
                    hh = (h % HP) * HD
                    dth = h // HP
                    d1 = exp_.tile([1, W], f32, tag="d1")
                    nc.sync.dma_start(out=d1[0:1, :], in_=dall[h : h + 1, :])
                    pb = pbp.tile([HD, W], f32, tag="pb")
                    nc.tensor.matmul(
                        pb[0:HD, :],
                        lhsT=ones_hd[0:1, :],
                        rhs=d1[0:1, :],
                        start=True, stop=True,
                    )
                    nc.vector.tensor_mul(
                        aoT[hh : hh + HD, dth, :],
                        aoT[hh : hh + HD, dth, :],
                        pb[0:HD, :],
                    )

        # =========== layernorm (transposed layout) ===========
        def ln_t(pre, out_t, g_sb, b_sb, lpp, lp, lst):
            acc = lp.tile([P, W], f32, tag="lnacc")
            nc.vector.tensor_add(acc[:, :], pre[:, 0, :], pre[:, 1, :])
            for d in range(2, DT):
                nc.vector.tensor_add(acc[:, :], acc[:, :], pre[:, d, :])
            sqa = lp.tile([P, W], f32, tag="lnsqa")
            nc.scalar.square(sqa[:, :], pre[:, 0, :])
            for d in range(1, DT):
                sqt = lp.tile([P, W], f32, tag="lnsqt")
                nc.scalar.square(sqt[:, :], pre[:, d, :])
                nc.vector.tensor_add(sqa[:, :], sqa[:, :], sqt[:, :])
            sums = lpp.tile([1, W], f32, tag="lnsums")
            nc.tensor.matmul(sums[0:1, :], lhsT=ones_p1[:, :],
                             rhs=acc[:, :], start=True, stop=True)
            sqs = lpp.tile([1, W], f32, tag="lnsqs")
            nc.tensor.matmul(sqs[0:1, :], lhsT=ones_p1[:, :],
                             rhs=sqa[:, :], start=True, stop=True)
            mu = lst.tile([1, W], f32, tag="lnmu")
            nc.vector.tensor_scalar_mul(mu[0:1, :], sums[0:1, :], 1.0 / D)
            ex2 = lst.tile([1, W], f32, tag="lnex2")
            nc.vector.tensor_scalar_mul(ex2[0:1, :], sqs[0:1, :], 1.0 / D)
            mu2 = lst.tile([1, W], f32, tag="lnmu2")
            nc.scalar.square(mu2[0:1, :], mu[0:1, :])
            var = lst.tile([1, W], f32, tag="lnvar")
            nc.vector.tensor_sub(var[0:1, :], ex2[0:1, :], mu2[0:1, :])
            sd = lst.tile([1, W], f32, tag="lnsd")
            nc.scalar.activation(out=sd[0:1, :], in_=var[0:1, :], func=AF.Sqrt,
                                 bias=eps_t[0:1, :], scale=1.0)
            rstd = lst.tile([1, W], f32, tag="lnrstd")
            nc.vector.reciprocal(rstd[0:1, :], sd[0:1, :])
            mub = lpp.tile([P, W], f32, tag="lnmub")
            nc.tensor.matmul(mub[:, :], lhsT=ones_1p[0:1, :],
                             rhs=mu[0:1, :], start=True, stop=True)
            rstdb = lpp.tile([P, W], f32, tag="lnrstdb")
            nc.tensor.matmul(rstdb[:, :], lhsT=ones_1p[0:1, :],
                             rhs=rstd[0:1, :], start=True, stop=True)
            for d in range(DT):
                t1 = lp.tile([P, W], f32, tag="lnt1")
                nc.vector.tensor_sub(t1[:, :], pre[:, d, :], mub[:, :])
                nc.vector.tensor_mul(out_t[:, d, :], t1[:, :], rstdb[:, :])
                if g_sb is not None:
                    nc.vector.tensor_scalar_mul(
                        out_t[:, d, :], out_t[:, d, :], g_sb[:, d : d + 1])
                if b_sb is not None:
                    nc.vector.tensor_scalar_add(
                        out_t[:, d, :], out_t[:, d, :], b_sb[:, d : d + 1])

        # =========== out-projection + residual + LN ===========
        def proj_resid_ln(owT, obsb, residT, g_sb, b_sb, out_t, out_b):
            """out_t: fp32 LN output; out_b: bf16 copy (or None)."""
            with tc.tile_pool(name="pr_w", bufs=2) as wp, \
                 tc.tile_pool(name="pr_t", bufs=2) as lp, \
                 tc.tile_pool(name="pr_st", bufs=1) as lst, \
                 tc.tile_pool(name="pr_pre", bufs=1) as prep, \
                 tc.tile_pool(name="pr_ps", bufs=2, space="PSUM") as psp, \
                 tc.tile_pool(name="pr_lnps", bufs=1, space="PSUM") as lpp:
                pre = prep.tile([P, DT, W], f32, tag="pre")
                G4 = min(4, DT)
                for dg in range(DT // G4):
                    wsl = wp.tile([P, DT, G4 * P], bf16, tag="prw")
                    nc.sync.dma_start(
                        out=wsl[:, :, :],
                        in_=owT[:, dg * G4 * P : (dg + 1) * G4 * P]
                        .rearrange("(t p) v -> p t v", p=P),
                    )
                    for j in range(G4):
                        d = dg * G4 + j
                        ps = psp.tile([P, W], f32, tag="prps")
                        for dt in range(DT):
                            nc.tensor.matmul(
                                ps[:, :], lhsT=wsl[:, dt, j * P : (j + 1) * P],
                                rhs=aoT[:, dt, :],
                                start=(dt == 0), stop=(dt == DT - 1),
                            )
                        if obsb is not None:
                            tmp = lp.tile([P, W], f32, tag="prtmp")
                            nc.scalar.activation(out=tmp[:, :], in_=ps[:, :],
                                                 func=AF.Identity,
                                                 bias=obsb[:, d : d + 1], scale=1.0)
                            nc.vector.tensor_add(pre[:, d, :], tmp[:, :],
                                                 residT[:, d, :])
                        else:
                            nc.vector.tensor_add(pre[:, d, :], ps[:, :],
                                                 residT[:, d, :])
                ln_t(pre, out_t, g_sb, b_sb, lpp, lp, lst)
                if out_b is not None:
                    for d in range(DT):
                        nc.scalar.activation(out=out_b[:, d, :],
                                             in_=out_t[:, d, :], func=AF.Copy)

        # ================= pipeline =================
        midp = es.enter_context(tc.tile_pool(name="mid", bufs=1))
        qT = midp.tile([P, DT, W], bf16)     # Q^T local (reused block2)
        aoT = midp.tile([P, DT, W], bf16)    # attention out^T (reused)
        x1T = midp.tile([P, DT, W], f32)     # x1 local fp32 (residual 2)
        x1b = midp.tile([P, DT, W], bf16)    # x1 local bf16 (matmul source)

        with tc.tile_pool(name="kv1", bufs=1) as kvp1:
            kT1sb = kvp1.tile([P, DT, S], bf16)
            v1sb = kvp1.tile([P, NT, H, HD1], bf16)
            with tc.tile_pool(name="xt1", bufs=1) as xtp1:
                xbf1 = xtp1.tile([P, DT, S], bf16)
                nc.sync.dma_start(out=xbf1[:, :, :],
                                  in_=xTb.rearrange("(t p) s -> p t s", p=P))
                xlb1 = xtp1.tile([P, DT, W], bf16)
                nc.sync.dma_start(out=xlb1[:, :, :],
                                  in_=xTlb.rearrange("(t p) s -> p t s", p=P))
                qkv_full(xbf1, xlb1, qkvwT1, kT1sb, v1sb, qkb1sb, vb1sb)

            with tc.tile_pool(name="xtl", bufs=1) as xtlp:
                xTlt = xtlp.tile([P, DT, W], f32)
                nc.sync.dma_start(out=xTlt[:, :, :],
                                  in_=xTl.rearrange("(t p) s -> p t s", p=P))
                if fl.m1:
                    with tc.tile_pool(name="m1p", bufs=1) as m1p:
                        m1sb = m1p.tile([P, NT, W], bf16)
                        nc.sync.dma_start(out=m1sb[:, :, :],
                                          in_=m1.rearrange("n p w -> p n w"))
                        attn_phase(kT1sb, v1sb, m1sb, False,
                                   causal=fl.causal)
                else:
                    attn_phase(kT1sb, v1sb, None, False)

                proj_resid_ln(owT1, ob1sb, xTlt, lns["g1"], lns["b1"],
                              x1T, x1b)

        x2p = es.enter_context(tc.tile_pool(name="x2p", bufs=1))
        x2T = x2p.tile([P, DT, W], f32)
        x2b = x2p.tile([P, DT, W], bf16)

        with tc.tile_pool(name="kv2", bufs=1) as kvp2:
            kT2sb = kvp2.tile([P, DT, S], bf16)
            v2sb = kvp2.tile([P, NT, H, HD1], bf16)
            # block-2 K/V from the core's own (interleaved) x1 columns;
            # gathered key n = g*W + w is token 4w+g, fine for unmasked
            # cross-attention (kb2 data is host-permuted to match).
            qkv_phase(x1b, x1b, qkvwT2, kT2sb, v2sb, qkb2sb, vb2sb,
                      kloc2, kg2, vloc2, vg2)

            attn_phase(kT2sb, v2sb, None, fl.kb2)

            proj_resid_ln(owT2, ob2sb, x1T, lns["g2"], lns["b2"], x2T, x2b)

        # ================= FFN =================
        with tc.tile_pool(name="ffh", bufs=1) as fhp, \
             tc.tile_pool(name="ffw", bufs=2) as wp, \
             tc.tile_pool(name="fft", bufs=2) as lp, \
             tc.tile_pool(name="ffst", bufs=1) as lst, \
             tc.tile_pool(name="ffpre", bufs=1) as prep:
            hT = fhp.tile([P, FT, W], bf16)
            G4 = min(4, DT)
            with tc.tile_pool(name="ffps1", bufs=2, space="PSUM") as psp:
                for fg in range(FT // G4):
                    wsl = wp.tile([P, DT, G4 * P], bf16, tag="f1w")
                    nc.sync.dma_start(
                        out=wsl[:, :, :],
                        in_=w1T[:, fg * G4 * P : (fg + 1) * G4 * P]
                        .rearrange("(t p) v -> p t v", p=P),
                    )
                    for j in range(G4):
                        f = fg * G4 + j
                        ps = psp.tile([P, W], f32, tag="f1ps")
                        for dt in range(DT):
                            nc.tensor.matmul(
                                ps[:, :], lhsT=wsl[:, dt, j * P : (j + 1) * P],
                                rhs=x2b[:, dt, :],
                                start=(dt == 0), stop=(dt == DT - 1),
                            )
                        if fb1sb is not None:
                            nc.scalar.activation(out=hT[:, f, :], in_=ps[:, :],
                                                 func=AF.Relu,
                                                 bias=fb1sb[:, f : f + 1], scale=1.0)
                        else:
                            nc.scalar.activation(out=hT[:, f, :], in_=ps[:, :],
                                                 func=AF.Relu)
            pre = prep.tile([P, DT, W], f32, tag="ffpre")
            with tc.tile_pool(name="ffps2", bufs=1, space="PSUM") as psq, \
                 tc.tile_pool(name="fflnps", bufs=1, space="PSUM") as lpp:
                for dg in range(DT // G4):
                    ps4 = []
                    for j in range(G4):
                        ps4j = psq.tile([P, W], f32, tag="f2ps%d" % j)
                        ps4.append(ps4j)
                    for ft in range(FT):
                        wsl = wp.tile([P, G4 * P], bf16, tag="f2w")
                        nc.sync.dma_start(
                            out=wsl[:, :],
                            in_=w2T[ft * P : (ft + 1) * P,
                                    dg * G4 * P : (dg + 1) * G4 * P],
                        )
                        for j in range(G4):
                            nc.tensor.matmul(
                                ps4[j][:, :],
                                lhsT=wsl[:, j * P : (j + 1) * P],
                                rhs=hT[:, ft, :],
                                start=(ft == 0), stop=(ft == FT - 1),
                            )
                    for j in range(G4):
                        d = dg * G4 + j
                        if fb2sb is not None:
                            tmp = lp.tile([P, W], f32, tag="f2tmp")
                            nc.scalar.activation(out=tmp[:, :], in_=ps4[j][:, :],
                                                 func=AF.Identity,
                                                 bias=fb2sb[:, d : d + 1], scale=1.0)
                            nc.vector.tensor_add(pre[:, d, :], tmp[:, :],
                                                 x2T[:, d, :])
                        else:
                            nc.vector.tensor_add(pre[:, d, :], ps4[j][:, :],
                                                 x2T[:, d, :])
                ln_t(pre, pre, lns["g3"], lns["b3"], lpp, lp, lst)
                for d in range(DT):
                    nc.sync.dma_start(out=out[d * P : (d + 1) * P, :],
                                      in_=pre[:, d, :])


def make_program(cfg, fl):
    from concourse import bacc
    import concourse.tile as tile

    nc = bacc.Bacc("TRN2", target_bir_lowering=False, debug=False,
                   num_devices=8)
    with tile.TileContext(nc) as tc:
        _build(nc, tc, cfg, fl)
    nc.compile()
    return nc


def prep_inputs(inputs, cfg):
    """Host-side data prep. Returns (in_maps, fl)."""
    import ml_dtypes

    B, S, D, H, DFF, W, NT = (cfg.B, cfg.S, cfg.D, cfg.H, cfg.DFF,
                              cfg.W, cfg.NT)
    f = np.float32
    bf = ml_dtypes.bfloat16
    x = np.asarray(inputs["x"], f)
    enc = np.asarray(inputs["enc_out"])
    trg = np.asarray(inputs["trg_mask"])
    fl = Flags()
    fl.qkb1 = bool(np.any(inputs["qkv_b1"]))
    fl.qkb2 = bool(np.any(inputs["qkv_b2"]))
    fl.vb1 = bool(np.any(np.asarray(inputs["qkv_b1"])[2 * D :]))
    fl.vb2 = bool(np.any(np.asarray(inputs["qkv_b2"])[2 * D :]))
    fl.ob1 = bool(np.any(inputs["out_b1"]))
    fl.ob2 = bool(np.any(inputs["out_b2"]))
    fl.fb1 = bool(np.any(inputs["ff_b1"]))
    fl.fb2 = bool(np.any(inputs["ff_b2"]))
    fl.g1 = not bool(np.all(np.asarray(inputs["ln1_g"]) == 1))
    fl.b1 = bool(np.any(inputs["ln1_b"]))
    fl.g2 = not bool(np.all(np.asarray(inputs["ln2_g"]) == 1))
    fl.b2 = bool(np.any(inputs["ln2_b"]))
    fl.g3 = not bool(np.all(np.asarray(inputs["ln3_g"]) == 1))
    fl.b3 = bool(np.any(inputs["ln3_b"]))
    fl.m1 = not bool(np.all(trg != 0))
    fl.kb2 = bool(np.any(enc == 0))
    # causal <=> no mask entries above the diagonal (so key > query can be
    # skipped statically); queries are interleaved (core r gets q = r::4)
    # which makes the per-column needed-key count uniform across cores.
    if fl.m1:
        # trg[b, 0, q, k]: entries with k > q are the strictly-upper triangle
        fl.causal = not bool(np.any(np.triu(trg[:, 0], 1)))

    shared = {
        "qkvwT1": np.ascontiguousarray(np.asarray(inputs["qkv_w1"], f).T).astype(bf),
        "qkvwT2": np.ascontiguousarray(np.asarray(inputs["qkv_w2"], f).T).astype(bf),
        "owT1": np.ascontiguousarray(np.asarray(inputs["out_w1"], f).T).astype(bf),
        "owT2": np.ascontiguousarray(np.asarray(inputs["out_w2"], f).T).astype(bf),
        "w1T": np.ascontiguousarray(np.asarray(inputs["ff_w1"], f).T).astype(bf),
        "w2T": np.ascontiguousarray(np.asarray(inputs["ff_w2"], f).T).astype(bf),
    }
    if fl.qkb1:
        shared["qkvb1"] = np.asarray(inputs["qkv_b1"], f)
    if fl.qkb2:
        shared["qkvb2"] = np.asarray(inputs["qkv_b2"], f)
    if fl.vb1:
        shared["vb1"] = np.broadcast_to(
            np.asarray(inputs["qkv_b1"], f)[2 * D :], (P, D)).copy()
    if fl.vb2:
        shared["vb2"] = np.broadcast_to(
            np.asarray(inputs["qkv_b2"], f)[2 * D :], (P, D)).copy()
    if fl.ob1:
        shared["ob1"] = np.asarray(inputs["out_b1"], f)
    if fl.ob2:
        shared["ob2"] = np.asarray(inputs["out_b2"], f)
    if fl.fb1:
        shared["fb1"] = np.asarray(inputs["ff_b1"], f)
    if fl.fb2:
        shared["fb2"] = np.asarray(inputs["ff_b2"], f)
    for nm, key, use in [("g1", "ln1_g", fl.g1), ("b1", "ln1_b", fl.b1),
                         ("g2", "ln2_g", fl.g2), ("b2", "ln2_b", fl.b2),
                         ("g3", "ln3_g", fl.g3), ("b3", "ln3_b", fl.b3)]:
        if use:
            shared[nm] = np.asarray(inputs[key], f)

    xTb_ = [np.ascontiguousarray(x[b].T) for b in range(B)]
    xTbf = [t.astype(bf) for t in xTb_]
    # token held at gathered sequence position n = g*W + w is 4*w + g
    # (core g's local column w is global query 4w+g)
    tok_of_n = 4 * (np.arange(S) % W) + (np.arange(S) // W)
    in_maps = []
    for c in range(8):
        b, r = c // 4, c % 4
        qidx = np.arange(r, S, 4)  # this core's (interleaved) queries
        m = dict(shared)
        m["xTb"] = xTbf[b]
        m["xTlb"] = np.ascontiguousarray(xTbf[b][:, qidx])
        m["xTl"] = np.ascontiguousarray(xTb_[b][:, qidx])
        if fl.m1:
            # m1[kt, i, j] = trg[0or b, 0, qidx[j], kt*P + i]  (0/1)
            tb = trg[b] if trg.shape[0] == B else trg[0]
            blk = tb[0, qidx, :]  # [W, S] (q, k)
            m["m1"] = np.ascontiguousarray(
                (blk.T != 0).astype(bf).reshape(NT, P, W))
        if fl.kb2:
            eb = enc[b, 0, 0, :]  # [S], indexed by token
            kbv = np.where(eb[tok_of_n] != 0, f(0.0), f(-1e20)).astype(f)
            m["kb2"] = kbv.reshape(NT, P, 1)
        in_maps.append(m)
    return in_maps, fl


def kernel_with_results(_run_kwargs=None, **inputs):
    from concourse.bass_utils import run_bass_kernel_spmd

    cfg = Cfg()
    x = np.asarray(inputs["x"])
    assert x.shape == (cfg.B, cfg.S, cfg.D), x.shape
    in_maps, fl = prep_inputs(inputs, cfg)
    nc = make_program(cfg, fl)
    res = run_bass_kernel_spmd(nc, in_maps, list(range(8)),
                               **(_run_kwargs or {}))
    y = np.empty((cfg.B, cfg.S, cfg.D), np.float32)
    for c in range(8):
        b, r = c // 4, c % 4
        y[b, r::4, :] = res.results[c]["out"].T
    return y, res


def kernel(**inputs):
    return kernel_with_results(**inputs)[0]


# revision 48
# speedup vs baseline: 1.0627x; 1.0615x over previous
"""Trainium2 Bass kernel for nn_DecoderLayer (dense transformer decoder layer).

Strategy (8 NeuronCores, full inputs in / full output out):
  - core c handles batch b = c//4 and query-quarter r = c%4 (rows [r*S/4, (r+1)*S/4)).
  - All matmul operands are bf16 (weights cast host-side, activations cast at
    the PSUM->SBUF copy points); PSUM accumulation stays fp32, as do residuals
    and LayerNorm statistics.
  - K^T and V(+ones column) live entirely in SBUF (no DRAM round-trip); x^T is
    loaded to SBUF once per block and sliced for the K/V/Q projections.
  - Attention per head: S^T[k, q] scores via PE (64-partition contraction),
    exp on ACT (1/8 scale fused, bf16 out), causal mask as multiplicative
    bf16 [128, W] tiles, softmax denominators free via a ones column in V,
    normalization deferred: unnormalized head outputs + per-head denominator
    rows are copied out of PSUM, one batched reciprocal [H, W] per block, then
    per-head PE ones-broadcast + vector multiply.
  - The single collective: bf16 AllGather of x1 (post-LN1) within each 4-core
    batch group; the Q2 projection is issued after it so it overlaps.
  - LayerNorm in transposed layout: cross-partition sums via ones-matmul on
    the PE, stats broadcast back to [128, W] via ones-matmul.
"""

import sys

if "/opt/trn_rl_repo" not in sys.path:
    sys.path.insert(0, "/opt/trn_rl_repo")

import numpy as np

P = 128
HD = 64
HD1 = HD + 1
EPS = 1e-5


class Cfg:
    def __init__(self, B=2, S=2048, D=1024, H=16, DFF=4096, use_collective=True,
                 fake_gather=False):
        self.B, self.S, self.D, self.H, self.DFF = B, S, D, H, DFF
        self.fake_gather = fake_gather
        self.W = S // 4            # local query rows per core
        self.DT = D // P           # feature-dim tiles
        self.NT = S // P           # sequence tiles (keys)
        self.FT = DFF // P         # ffn hidden tiles
        self.HP = P // HD          # heads per partition-tile (2)
        self.NCH = max(1, S // 512)   # n-chunks for K-orientation matmuls
        self.NCW = S // self.NCH      # n-chunk width (<=512)
        self.VCW = min(512, D)        # v-dout chunk width
        self.VCN = D // self.VCW
        self.KTG = 2                  # k-tiles per exp group
        self.NG = self.NT // self.KTG
        self.use_collective = use_collective
        assert D == H * HD
        assert self.W % P == 0 and D % P == 0 and DFF % P == 0 and S % P == 0
        assert self.NT % self.KTG == 0


class Flags:
    def __init__(self):
        self.qkb1 = self.vb1 = self.ob1 = False
        self.qkb2 = self.vb2 = self.ob2 = False
        self.fb1 = self.fb2 = False
        self.g1 = self.b1 = self.g2 = self.b2 = self.g3 = self.b3 = False
        self.m1 = True      # trg mask multiplicative tiles
        self.kb2 = False    # enc mask additive per-k bias
        self.causal = False  # trg mask is lower-triangular -> column skip


def _build(nc, tc, cfg, fl):
    import concourse.bass as bass
    import concourse.mybir as mybir
    import concourse.tile as tile  # noqa: F401
    from contextlib import ExitStack

    AF = mybir.ActivationFunctionType
    f32 = mybir.dt.float32
    bf16 = mybir.dt.bfloat16

    B, S, D, H, DFF = cfg.B, cfg.S, cfg.D, cfg.H, cfg.DFF
    W, DT, NT, FT, HP = cfg.W, cfg.DT, cfg.NT, cfg.FT, cfg.HP
    NCH, NCW, VCW, VCN = cfg.NCH, cfg.NCW, cfg.VCW, cfg.VCN
    KTG, NG = cfg.KTG, cfg.NG
    HPC = VCW // HD  # heads per v-chunk

    # ---------------- DRAM parameters ----------------
    def din(name, shape, dt=f32):
        return nc.dram_tensor(name, shape, dt, kind="ExternalInput").ap()

    xTb = din("xTb", [D, S], bf16)       # full x^T (block-1 K/V source)
    xTlb = din("xTlb", [D, W], bf16)     # interleaved queries, bf16 (Q source)
    xTl = din("xTl", [D, W])             # interleaved queries, fp32 (residual)
    qkvwT1 = din("qkvwT1", [D, 3 * D], bf16)
    qkvwT2 = din("qkvwT2", [D, 3 * D], bf16)
    owT1 = din("owT1", [D, D], bf16)
    owT2 = din("owT2", [D, D], bf16)
    w1T = din("w1T", [D, DFF], bf16)
    w2T = din("w2T", [DFF, D], bf16)
    m1 = din("m1", [NT, P, W], bf16) if fl.m1 else None
    kb2 = din("kb2", [NT, P, 1]) if fl.kb2 else None
    qkvb1 = din("qkvb1", [3 * D]) if fl.qkb1 else None
    qkvb2 = din("qkvb2", [3 * D]) if fl.qkb2 else None
    vb1 = din("vb1", [P, D]) if fl.vb1 else None
    vb2 = din("vb2", [P, D]) if fl.vb2 else None
    ob1 = din("ob1", [D]) if fl.ob1 else None
    ob2 = din("ob2", [D]) if fl.ob2 else None
    fb1d = din("fb1", [DFF]) if fl.fb1 else None
    fb2d = din("fb2", [D]) if fl.fb2 else None
    lnp = {}
    for nm, use in [("g1", fl.g1), ("b1", fl.b1), ("g2", fl.g2),
                    ("b2", fl.b2), ("g3", fl.g3), ("b3", fl.b3)]:
        lnp[nm] = din(nm, [D]) if use else None
    out = nc.dram_tensor("out", [D, W], f32, kind="ExternalOutput").ap()

    NTQ = W // P          # token tiles in the local quarter
    KL = D * W            # K staging elements
    VL = NTQ * P * H * HD1  # V staging elements
    HX = H * HD1

    es = ExitStack()
    with es:
        dramp = es.enter_context(tc.tile_pool(name="dram", bufs=1, space="DRAM"))
        KH = KL // 2          # K staging elements per head-half (dth 0-3 / 4-7)
        VH = VL // 2          # V staging elements per head-half (h 0-7 / 8-15)
        klocs = [dramp.tile([KH], bf16, name=f"kloc{i}") for i in range(2)]
        kgs = [dramp.tile([4 * KH], bf16, name=f"kg{i}") for i in range(2)]
        vlocs = [dramp.tile([VH], bf16, name=f"vloc{i}") for i in range(2)]
        vgs = [dramp.tile([4 * VH], bf16, name=f"vg{i}") for i in range(2)]

        const = es.enter_context(tc.tile_pool(name="const", bufs=1))
        ones_p1 = const.tile([P, 1], f32)
        nc.vector.memset(ones_p1[:, :], 1.0)
        ones_1p = const.tile([1, P], f32)
        nc.vector.memset(ones_1p[0:1, :], 1.0)
        ones_hd = const.tile([P, HD], f32)
        nc.vector.memset(ones_hd[:, :], 1.0)
        eps_t = const.tile([1, 1], f32)
        nc.vector.memset(eps_t[0:1, :], EPS)

        def ldvec(dram_vec, n_tiles, name):
            """[D]-style vector -> [P, n_tiles] sbuf tile (per-partition slices)."""
            t = const.tile([P, n_tiles], f32, tag=name)
            nc.sync.dma_start(
                out=t[:, :],
                in_=dram_vec.rearrange("(t p) -> p t", p=P),
            )
            return t

        qkb1sb = ldvec(qkvb1[0 : 2 * D], 2 * DT, "qkb1") if fl.qkb1 else None
        qkb2sb = ldvec(qkvb2[0 : 2 * D], 2 * DT, "qkb2") if fl.qkb2 else None
        ob1sb = ldvec(ob1, DT, "ob1") if fl.ob1 else None
        ob2sb = ldvec(ob2, DT, "ob2") if fl.ob2 else None
        fb1sb = ldvec(fb1d, FT, "fb1") if fl.fb1 else None
        fb2sb = ldvec(fb2d, DT, "fb2") if fl.fb2 else None
        lns = {k: (ldvec(v, DT, "ln" + k) if v is not None else None)
               for k, v in lnp.items()}
        vb1sb = None
        if fl.vb1:
            vb1sb = const.tile([P, D], f32, tag="vb1")
            nc.sync.dma_start(out=vb1sb[:, :], in_=vb1[:, :])
        vb2sb = None
        if fl.vb2:
            vb2sb = const.tile([P, D], f32, tag="vb2")
            nc.sync.dma_start(out=vb2sb[:, :], in_=vb2[:, :])
        kb2sb = None
        if fl.kb2:
            kb2sb = const.tile([P, NT], f32, tag="kb2")
            nc.sync.dma_start(out=kb2sb[:, :], in_=kb2.rearrange("n p o -> p (n o)"))

        # qT/aoT are assigned later (mid pool); closures below late-bind.
        qT = aoT = None

        # =========== QKV projection phase (local quarter + exchange) ===========
        def gather(loc, g_out):
            if cfg.fake_gather:
                n = loc.shape[0]
                for g in range(4):
                    nc.sync.dma_start(out=g_out[g * n : (g + 1) * n],
                                      in_=loc[:])
            else:
                nc.gpsimd.collective_compute(
                    "AllGather",
                    bass.mybir.AluOpType.bypass,
                    replica_groups=[[0, 1, 2, 3], [4, 5, 6, 7]],
                    ins=[loc[:]],
                    outs=[g_out[:]],
                )

        def qkv_full(xbf, xq, wT, kTsb, vsb, qkb, vbsb):
            """Block-1 path: K^T/V for the FULL sequence from resident x^T
            (redundant across the 4-core group, but keeps the PE busy with
            no collective on the critical path)."""
            with tc.tile_pool(name="qkf_w", bufs=2) as wp, \
                 tc.tile_pool(name="qkf_st", bufs=2) as stp, \
                 tc.tile_pool(name="qkf_ps", bufs=2, space="PSUM") as psp:
                for dk in range(DT):
                    wk = wp.tile([P, DT, P], bf16, tag="wk")
                    nc.sync.dma_start(
                        out=wk[:, :, :],
                        in_=wT[:, D + dk * P : D + (dk + 1) * P]
                        .rearrange("(t p) v -> p t v", p=P),
                    )
                    for nch in range(NCH):
                        ps = psp.tile([P, NCW], f32, tag="kps")
                        for dt in range(DT):
                            nc.tensor.matmul(
                                ps[:, :],
                                lhsT=wk[:, dt, :],
                                rhs=xbf[:, dt, nch * NCW : (nch + 1) * NCW],
                                start=(dt == 0),
                                stop=(dt == DT - 1),
                            )
                        if qkb is not None:
                            nc.scalar.activation(
                                out=kTsb[:, dk, nch * NCW : (nch + 1) * NCW],
                                in_=ps[:, :], func=AF.Identity,
                                bias=qkb[:, DT + dk : DT + dk + 1], scale=1.0,
                            )
                        else:
                            nc.scalar.activation(
                                out=kTsb[:, dk, nch * NCW : (nch + 1) * NCW],
                                in_=ps[:, :], func=AF.Copy,
                            )
                nc.vector.memset(vsb[:, :, :, HD:HD1], 1.0)
                for vc in range(VCN):
                    wv = wp.tile([P, DT, VCW], bf16, tag="wv")
                    nc.sync.dma_start(
                        out=wv[:, :, :],
                        in_=wT[:, 2 * D + vc * VCW : 2 * D + (vc + 1) * VCW]
                        .rearrange("(t p) v -> p t v", p=P),
                    )
                    for nt in range(NT):
                        ps = psp.tile([P, VCW], f32, tag="vps")
                        for dt in range(DT):
                            nc.tensor.matmul(
                                ps[:, :],
                                lhsT=xbf[:, dt, nt * P : (nt + 1) * P],
                                rhs=wv[:, dt, :],
                                start=(dt == 0),
                                stop=(dt == DT - 1),
                            )
                        vdst = vsb[:, nt, vc * HPC : (vc + 1) * HPC, 0:HD]
                        if vbsb is not None:
                            st = stp.tile([P, HPC, HD], f32, tag="vst")
                            nc.vector.tensor_add(
                                st[:, :, :],
                                ps.rearrange("p (h d) -> p h d", d=HD),
                                vbsb[:, vc * VCW : (vc + 1) * VCW].rearrange(
                                    "p (h d) -> p h d", d=HD),
                            )
                            nc.scalar.activation(
                                out=vdst, in_=st[:, :, :], func=AF.Copy)
                        else:
                            nc.scalar.activation(
                                out=vdst,
                                in_=ps.rearrange("p (h d) -> p h d", d=HD),
                                func=AF.Copy,
                            )
                for dq in range(DT):
                    wq = wp.tile([P, DT, P], bf16, tag="wq")
                    nc.sync.dma_start(
                        out=wq[:, :, :],
                        in_=wT[:, dq * P : (dq + 1) * P]
                        .rearrange("(t p) v -> p t v", p=P),
                    )
                    ps = psp.tile([P, W], f32, tag="qps")
                    for dt in range(DT):
                        nc.tensor.matmul(
                            ps[:, :],
                            lhsT=wq[:, dt, :],
                            rhs=xq[:, dt, :],
                            start=(dt == 0),
                            stop=(dt == DT - 1),
                        )
                    if qkb is not None:
                        nc.scalar.activation(
                            out=qT[:, dq, :], in_=ps[:, :], func=AF.Identity,
                            bias=qkb[:, dq : dq + 1], scale=1.0,
                        )
                    else:
                        nc.scalar.activation(
                            out=qT[:, dq, :], in_=ps[:, :], func=AF.Copy,
                        )

        def qkv_phase(xqn, xq, wT, kTsb, vsb, qkb, vbsb):
            """K/V projected from the LOCAL token quarter xqn [P, DT, W],
            all-gathered within the 4-core batch group in HEAD-HALVES
            (K[h0-7] -> V[h0-7] -> K[h8-15] -> V[h8-15]) so attention on the
            first heads overlaps the later gathers. Q^T from xq."""
            H2 = H // 2
            DTH = DT // 2
            va = (H2 - 1) // HPC  # vc index that completes head-half 0
            with tc.tile_pool(name="qkv_wall", bufs=1) as wallp, \
                 tc.tile_pool(name="qkv_loc", bufs=1) as locp, \
                 tc.tile_pool(name="qkv_st", bufs=2) as stp, \
                 tc.tile_pool(name="qkv_ps", bufs=2, space="PSUM") as psp:
                # Prefetch all weights up front: DMA issued during a gather
                # gets starved behind the collective's transfers.
                wkall = wallp.tile([P, DT, D], bf16, tag="wkall")
                nc.sync.dma_start(
                    out=wkall[:, :, :],
                    in_=wT[:, D : 2 * D].rearrange("(t p) v -> p t v", p=P),
                )
                wvall = wallp.tile([P, DT, D], bf16, tag="wvall")
                nc.sync.dma_start(
                    out=wvall[:, :, :],
                    in_=wT[:, 2 * D : 3 * D].rearrange("(t p) v -> p t v", p=P),
                )
                wqall = wallp.tile([P, DT, D], bf16, tag="wqall")
                nc.sync.dma_start(
                    out=wqall[:, :, :],
                    in_=wT[:, 0:D].rearrange("(t p) v -> p t v", p=P),
                )
                klocsb = locp.tile([P, DT, W], bf16, tag="kloc")
                vlocsb = locp.tile([P, NTQ, H, HD1], bf16, tag="vloc")
                nc.vector.memset(vlocsb[:, :, :, HD:HD1], 1.0)

                def kpart(half):
                    for dk in range(half * DTH, (half + 1) * DTH):
                        ps = psp.tile([P, W], f32, tag="kps")
                        for dt in range(DT):
                            nc.tensor.matmul(
                                ps[:, :],
                                lhsT=wkall[:, dt, dk * P : (dk + 1) * P],
                                rhs=xqn[:, dt, :],
                                start=(dt == 0),
                                stop=(dt == DT - 1),
                            )
                        if qkb is not None:
                            nc.scalar.activation(
                                out=klocsb[:, dk, :], in_=ps[:, :],
                                func=AF.Identity,
                                bias=qkb[:, DT + dk : DT + dk + 1], scale=1.0,
                            )
                        else:
                            nc.scalar.activation(
                                out=klocsb[:, dk, :], in_=ps[:, :],
                                func=AF.Copy,
                            )
                    nc.sync.dma_start(
                        out=klocs[half][:].rearrange("(t p w) -> p t w",
                                                     p=P, w=W),
                        in_=klocsb[:, half * DTH : (half + 1) * DTH, :],
                    )
                    gather(klocs[half], kgs[half])

                def vpart(vc):
                    for nt in range(NTQ):
                        ps = psp.tile([P, VCW], f32, tag="vps")
                        for dt in range(DT):
                            nc.tensor.matmul(
                                ps[:, :],
                                lhsT=xqn[:, dt, nt * P : (nt + 1) * P],
                                rhs=wvall[:, dt, vc * VCW : (vc + 1) * VCW],
                                start=(dt == 0),
                                stop=(dt == DT - 1),
                            )
                        vdst = vlocsb[:, nt, vc * HPC : (vc + 1) * HPC, 0:HD]
                        if vbsb is not None:
                            st = stp.tile([P, HPC, HD], f32, tag="vst")
                            nc.vector.tensor_add(
                                st[:, :, :],
                                ps.rearrange("p (h d) -> p h d", d=HD),
                                vbsb[:, vc * VCW : (vc + 1) * VCW].rearrange(
                                    "p (h d) -> p h d", d=HD),
                            )
                            nc.scalar.activation(
                                out=vdst, in_=st[:, :, :], func=AF.Copy)
                        else:
                            nc.scalar.activation(
                                out=vdst,
                                in_=ps.rearrange("p (h d) -> p h d", d=HD),
                                func=AF.Copy,
                            )

                def vstage(half):
                    nc.sync.dma_start(
                        out=vlocs[half][:].rearrange("(n p x) -> p n x",
                                                     p=P, x=H2 * HD1),
                        in_=vlocsb[:, :, half * H2 : (half + 1) * H2, :]
                        .rearrange("p n h d -> p n (h d)"),
                    )
                    gather(vlocs[half], vgs[half])

                kpart(0)
                for vc in range(va + 1):
                    vpart(vc)
                vstage(0)
                kpart(1)
                for vc in range(va + 1, VCN):
                    vpart(vc)
                vstage(1)
                # ---- Q^T local [D, W] (overlaps the gathers) ----
                for dq in range(DT):
                    ps = psp.tile([P, W], f32, tag="qps")
                    for dt in range(DT):
                        nc.tensor.matmul(
                            ps[:, :],
                            lhsT=wqall[:, dt, dq * P : (dq + 1) * P],
                            rhs=xq[:, dt, :],
                            start=(dt == 0),
                            stop=(dt == DT - 1),
                        )
                    if qkb is not None:
                        nc.scalar.activation(
                            out=qT[:, dq, :], in_=ps[:, :], func=AF.Identity,
                            bias=qkb[:, dq : dq + 1], scale=1.0,
                        )
                    else:
                        nc.scalar.activation(
                            out=qT[:, dq, :], in_=ps[:, :], func=AF.Copy,
                        )
                # ---- land gathered K^T / V halves into SBUF ----
                for half in range(2):
                    for g in range(4):
                        nc.sync.dma_start(
                            out=kTsb[:, half * DTH : (half + 1) * DTH,
                                     g * W : (g + 1) * W],
                            in_=kgs[half][g * KH : (g + 1) * KH]
                            .rearrange("(t p w) -> p t w", p=P, w=W),
                        )
                        eng = nc.sync if g % 2 == 0 else nc.scalar
                        eng.dma_start(
                            out=vsb[:, g * NTQ : (g + 1) * NTQ,
                                    half * H2 : (half + 1) * H2, :]
                            .rearrange("p n h d -> p n (h d)"),
                            in_=vgs[half][g * VH : (g + 1) * VH]
                            .rearrange("(n p x) -> p n x", p=P, x=H2 * HD1),
                        )

        # =========== attention phase ===========
        def attn_phase(kTsb, vsb, m1sb, use_kb2, causal=False):
            """causal: queries are interleaved (local col j = global query
            4j+r), so key-tile group g is only needed by columns
            >= KTG*32*g — uniformly across cores. Skip the earlier columns."""
            def jg_of(g):
                return min(W, KTG * 32 * g) if causal else 0

            with tc.tile_pool(name="at_ex", bufs=2) as exp_, \
                 tc.tile_pool(name="at_dn", bufs=1) as dnp, \
                 tc.tile_pool(name="at_ps", bufs=2, space="PSUM") as psp, \
                 tc.tile_pool(name="at_po", bufs=2, space="PSUM") as pop, \
                 tc.tile_pool(name="at_pb", bufs=2, space="PSUM") as pbp:
                dall = dnp.tile([P, W], f32, tag="dall")
                for h in range(H):
                    hh = (h % HP) * HD  # partition base shared with q_h
                    dth = h // HP
                    q_h = qT[hh : hh + HD, dth, :]
                    po = pop.tile([P, W], f32, tag="po")
                    ng = sum(1 for g in range(NG) if jg_of(g) < W)
                    for g in range(ng):
                        jg = jg_of(g)
                        ps = psp.tile([P, KTG, W], f32, tag="sc")
                        for o in range(KTG):
                            kt = g * KTG + o
                            nc.tensor.matmul(
                                ps[:, o, jg:W],
                                lhsT=kTsb[hh : hh + HD, dth,
                                          kt * P : (kt + 1) * P],
                                rhs=q_h[:, jg:W],
                                start=True,
                                stop=True,
                            )
                        ex = exp_.tile([P, KTG, W], bf16, tag="ex")
                        if use_kb2:
                            for o in range(KTG):
                                kt = g * KTG + o
                                nc.scalar.activation(
                                    out=ex[:, o, jg:W], in_=ps[:, o, jg:W],
                                    func=AF.Exp,
                                    bias=kb2sb[:, kt : kt + 1],
                                    scale=1.0 / np.sqrt(HD),
                                )
                        else:
                            nc.scalar.activation(
                                out=ex[:, :, jg:W], in_=ps[:, :, jg:W],
                                func=AF.Exp,
                                scale=1.0 / np.sqrt(HD),
                            )
                        if m1sb is not None:
                            nc.vector.tensor_mul(
                                ex[:, :, jg:W], ex[:, :, jg:W],
                                m1sb[:, g * KTG : (g + 1) * KTG, jg:W],
                            )
                        for o in range(KTG):
                            kt = g * KTG + o
                            nc.tensor.matmul(
                                po[0:HD1, jg:W],
                                lhsT=vsb[:, kt, h, :],
                                rhs=ex[:, o, jg:W],
                                start=(g == 0 and o == 0),
                                stop=(g == ng - 1 and o == KTG - 1),
                            )
                    # unnormalized head output + denominator row out of PSUM
                    nc.scalar.activation(
                        out=aoT[hh : hh + HD, dth, :], in_=po[0:HD, :],
                        func=AF.Copy,
                    )
                    s64 = exp_.tile([HD1, W], f32, tag="s64")
                    nc.vector.tensor_copy(s64[HD:HD1, :], po[HD:HD1, :])
                    nc.sync.dma_start(out=dall[h : h + 1, :],
                                      in_=s64[HD:HD1, :])
                # batched softmax normalization
                nc.vector.reciprocal(dall[0:H, :], dall[0:H, :])
                for h in range(H):
                    hh = (h % HP) * HD
                    dth = h // HP
                    d1 = exp_.tile([1, W], f32, tag="d1")
                    nc.sync.dma_start(out=d1[0:1, :], in_=dall[h : h + 1, :])
                    pb = pbp.tile([HD, W], f32, tag="pb")
                    nc.tensor.matmul(
                        pb[0:HD, :],
                        lhsT=ones_hd[0:1, :],
                        rhs=d1[0:1, :],
                        start=True, stop=True,
                    )
                    nc.vector.tensor_mul(
                        aoT[hh : hh + HD, dth, :],
                        aoT[hh : hh + HD, dth, :],
                        pb[0:HD, :],
                    )

        # =========== layernorm (transposed layout) ===========
        def ln_t(pre, out_t, g_sb, b_sb, lpp, lp, lst):
            acc = lp.tile([P, W], f32, tag="lnacc")
            nc.vector.tensor_add(acc[:, :], pre[:, 0, :], pre[:, 1, :])
            for d in range(2, DT):
                nc.vector.tensor_add(acc[:, :], acc[:, :], pre[:, d, :])
            sqa = lp.tile([P, W], f32, tag="lnsqa")
            nc.scalar.square(sqa[:, :], pre[:, 0, :])
            for d in range(1, DT):
                sqt = lp.tile([P, W], f32, tag="lnsqt")
                nc.scalar.square(sqt[:, :], pre[:, d, :])
                nc.vector.tensor_add(sqa[:, :], sqa[:, :], sqt[:, :])
            sums = lpp.tile([1, W], f32, tag="lnsums")
            nc.tensor.matmul(sums[0:1, :], lhsT=ones_p1[:, :],
                             rhs=acc[:, :], start=True, stop=True)
            sqs = lpp.tile([1, W], f32, tag="lnsqs")
            nc.tensor.matmul(sqs[0:1, :], lhsT=ones_p1[:, :],
                             rhs=sqa[:, :], start=True, stop=True)
            mu = lst.tile([1, W], f32, tag="lnmu")
            nc.vector.tensor_scalar_mul(mu[0:1, :], sums[0:1, :], 1.0 / D)
            ex2 = lst.tile([1, W], f32, tag="lnex2")
            nc.vector.tensor_scalar_mul(ex2[0:1, :], sqs[0:1, :], 1.0 / D)
            mu2 = lst.tile([1, W], f32, tag="lnmu2")
            nc.scalar.square(mu2[0:1, :], mu[0:1, :])
            var = lst.tile([1, W], f32, tag="lnvar")
            nc.vector.tensor_sub(var[0:1, :], ex2[0:1, :], mu2[0:1, :])
            sd = lst.tile([1, W], f32, tag="lnsd")
            nc.scalar.activation(out=sd[0:1, :], in_=var[0:1, :], func=AF.Sqrt,
                                 bias=eps_t[0:1, :], scale=1.0)
            rstd = lst.tile([1, W], f32, tag="lnrstd")
            nc.vector.reciprocal(rstd[0:1, :], sd[0:1, :])
            mub = lpp.tile([P, W], f32, tag="lnmub")
            nc.tensor.matmul(mub[:, :], lhsT=ones_1p[0:1, :],
                             rhs=mu[0:1, :], start=True, stop=True)
            rstdb = lpp.tile([P, W], f32, tag="lnrstdb")
            nc.tensor.matmul(rstdb[:, :], lhsT=ones_1p[0:1, :],
                             rhs=rstd[0:1, :], start=True, stop=True)
            for d in range(DT):
                t1 = lp.tile([P, W], f32, tag="lnt1")
                nc.vector.tensor_sub(t1[:, :], pre[:, d, :], mub[:, :])
                nc.vector.tensor_mul(out_t[:, d, :], t1[:, :], rstdb[:, :])
                if g_sb is not None:
                    nc.vector.tensor_scalar_mul(
                        out_t[:, d, :], out_t[:, d, :], g_sb[:, d : d + 1])
                if b_sb is not None:
                    nc.vector.tensor_scalar_add(
                        out_t[:, d, :], out_t[:, d, :], b_sb[:, d : d + 1])

        # =========== out-projection + residual + LN ===========
        def proj_resid_ln(owT, obsb, residT, g_sb, b_sb, out_t, out_b):
            """out_t: fp32 LN output; out_b: bf16 copy (or None)."""
            with tc.tile_pool(name="pr_w", bufs=2) as wp, \
                 tc.tile_pool(name="pr_t", bufs=2) as lp, \
                 tc.tile_pool(name="pr_st", bufs=1) as lst, \
                 tc.tile_pool(name="pr_pre", bufs=1) as prep, \
                 tc.tile_pool(name="pr_ps", bufs=2, space="PSUM") as psp, \
                 tc.tile_pool(name="pr_lnps", bufs=1, space="PSUM") as lpp:
                pre = prep.tile([P, DT, W], f32, tag="pre")
                G4 = min(4, DT)
                for dg in range(DT // G4):
                    wsl = wp.tile([P, DT, G4 * P], bf16, tag="prw")
                    nc.sync.dma_start(
                        out=wsl[:, :, :],
                        in_=owT[:, dg * G4 * P : (dg + 1) * G4 * P]
                        .rearrange("(t p) v -> p t v", p=P),
                    )
                    for j in range(G4):
                        d = dg * G4 + j
                        ps = psp.tile([P, W], f32, tag="prps")
                        for dt in range(DT):
                            nc.tensor.matmul(
                                ps[:, :], lhsT=wsl[:, dt, j * P : (j + 1) * P],
                                rhs=aoT[:, dt, :],
                                start=(dt == 0), stop=(dt == DT - 1),
                            )
                        if obsb is not None:
                            tmp = lp.tile([P, W], f32, tag="prtmp")
                            nc.scalar.activation(out=tmp[:, :], in_=ps[:, :],
                                                 func=AF.Identity,
                                                 bias=obsb[:, d : d + 1], scale=1.0)
                            nc.vector.tensor_add(pre[:, d, :], tmp[:, :],
                                                 residT[:, d, :])
                        else:
                            nc.vector.tensor_add(pre[:, d, :], ps[:, :],
                                                 residT[:, d, :])
                ln_t(pre, out_t, g_sb, b_sb, lpp, lp, lst)
                if out_b is not None:
                    for d in range(DT):
                        nc.scalar.activation(out=out_b[:, d, :],
                                             in_=out_t[:, d, :], func=AF.Copy)

        # ================= pipeline =================
        midp = es.enter_context(tc.tile_pool(name="mid", bufs=1))
        qT = midp.tile([P, DT, W], bf16)     # Q^T local (reused block2)
        aoT = midp.tile([P, DT, W], bf16)    # attention out^T (reused)
        x1T = midp.tile([P, DT, W], f32)     # x1 local fp32 (residual 2)
        x1b = midp.tile([P, DT, W], bf16)    # x1 local bf16 (matmul source)

        with tc.tile_pool(name="kv1", bufs=1) as kvp1:
            kT1sb = kvp1.tile([P, DT, S], bf16)
            v1sb = kvp1.tile([P, NT, H, HD1], bf16)
            with tc.tile_pool(name="xt1", bufs=1) as xtp1:
                xbf1 = xtp1.tile([P, DT, S], bf16)
                nc.sync.dma_start(out=xbf1[:, :, :],
                                  in_=xTb.rearrange("(t p) s -> p t s", p=P))
                xlb1 = xtp1.tile([P, DT, W], bf16)
                nc.sync.dma_start(out=xlb1[:, :, :],
                                  in_=xTlb.rearrange("(t p) s -> p t s", p=P))
                qkv_full(xbf1, xlb1, qkvwT1, kT1sb, v1sb, qkb1sb, vb1sb)

            with tc.tile_pool(name="xtl", bufs=1) as xtlp:
                xTlt = xtlp.tile([P, DT, W], f32)
                nc.sync.dma_start(out=xTlt[:, :, :],
                                  in_=xTl.rearrange("(t p) s -> p t s", p=P))
                if fl.m1:
                    with tc.tile_pool(name="m1p", bufs=1) as m1p:
                        m1sb = m1p.tile([P, NT, W], bf16)
                        nc.sync.dma_start(out=m1sb[:, :, :],
                                          in_=m1.rearrange("n p w -> p n w"))
                        attn_phase(kT1sb, v1sb, m1sb, False,
                                   causal=fl.causal)
                else:
                    attn_phase(kT1sb, v1sb, None, False)

                proj_resid_ln(owT1, ob1sb, xTlt, lns["g1"], lns["b1"],
                              x1T, x1b)

        x2p = es.enter_context(tc.tile_pool(name="x2p", bufs=1))
        x2T = x2p.tile([P, DT, W], f32)
        x2b = x2p.tile([P, DT, W], bf16)

        with tc.tile_pool(name="kv2", bufs=1) as kvp2:
            kT2sb = kvp2.tile([P, DT, S], bf16)
            v2sb = kvp2.tile([P, NT, H, HD1], bf16)
            # block-2 K/V from the core's own (interleaved) x1 columns;
            # gathered key n = g*W + w is token 4w+g, fine for unmasked
            # cross-attention (kb2 data is host-permuted to match).
            qkv_phase(x1b, x1b, qkvwT2, kT2sb, v2sb, qkb2sb, vb2sb)

            attn_phase(kT2sb, v2sb, None, fl.kb2)

            proj_resid_ln(owT2, ob2sb, x1T, lns["g2"], lns["b2"], x2T, x2b)

        # ================= FFN =================
        with tc.tile_pool(name="ffh", bufs=1) as fhp, \
             tc.tile_pool(name="ffw", bufs=2) as wp, \
             tc.tile_pool(name="fft", bufs=2) as lp, \
             tc.tile_pool(name="ffst", bufs=1) as lst, \
             tc.tile_pool(name="ffpre", bufs=1) as prep:
            hT = fhp.tile([P, FT, W], bf16)
            G4 = min(4, DT)
            with tc.tile_pool(name="ffps1", bufs=2, space="PSUM") as psp:
                for fg in range(FT // G4):
                    wsl = wp.tile([P, DT, G4 * P], bf16, tag="f1w")
                    nc.sync.dma_start(
                        out=wsl[:, :, :],
                        in_=w1T[:, fg * G4 * P : (fg + 1) * G4 * P]
                        .rearrange("(t p) v -> p t v", p=P),
                    )
                    for j in range(G4):
                        f = fg * G4 + j
                        ps = psp.tile([P, W], f32, tag="f1ps")
                        for dt in range(DT):
                            nc.tensor.matmul(
                                ps[:, :], lhsT=wsl[:, dt, j * P : (j + 1) * P],
                                rhs=x2b[:, dt, :],
                                start=(dt == 0), stop=(dt == DT - 1),
                            )
                        if fb1sb is not None:
                            nc.scalar.activation(out=hT[:, f, :], in_=ps[:, :],
                                                 func=AF.Relu,
                                                 bias=fb1sb[:, f : f + 1], scale=1.0)
                        else:
                            nc.scalar.activation(out=hT[:, f, :], in_=ps[:, :],
                                                 func=AF.Relu)
            pre = prep.tile([P, DT, W], f32, tag="ffpre")
            with tc.tile_pool(name="ffps2", bufs=1, space="PSUM") as psq, \
                 tc.tile_pool(name="fflnps", bufs=1, space="PSUM") as lpp:
                for dg in range(DT // G4):
                    ps4 = []
                    for j in range(G4):
                        ps4j = psq.tile([P, W], f32, tag="f2ps%d" % j)
                        ps4.append(ps4j)
                    for ft in range(FT):
                        wsl = wp.tile([P, G4 * P], bf16, tag="f2w")
                        nc.sync.dma_start(
                            out=wsl[:, :],
                            in_=w2T[ft * P : (ft + 1) * P,
                                    dg * G4 * P : (dg + 1) * G4 * P],
                        )
                        for j in range(G4):
                            nc.tensor.matmul(
                                ps4[j][:, :],
                                lhsT=wsl[:, j * P : (j + 1) * P],
                                rhs=hT[:, ft, :],
                                start=(ft == 0), stop=(ft == FT - 1),
                            )
                    for j in range(G4):
                        d = dg * G4 + j
                        if fb2sb is not None:
                            tmp = lp.tile([P, W], f32, tag="f2tmp")
                            nc.scalar.activation(out=tmp[:, :], in_=ps4[j][:, :],
                                                 func=AF.Identity,
                                                 bias=fb2sb[:, d : d + 1], scale=1.0)
                            nc.vector.tensor_add(pre[:, d, :], tmp[:, :],
                                                 x2T[:, d, :])
                        else:
                            nc.vector.tensor_add(pre[:, d, :], ps4[j][:, :],
                                                 x2T[:, d, :])
                ln_t(pre, pre, lns["g3"], lns["b3"], lpp, lp, lst)
                for d in range(DT):
                    nc.sync.dma_start(out=out[d * P : (d + 1) * P, :],
                                      in_=pre[:, d, :])


def make_program(cfg, fl):
    from concourse import bacc
    import concourse.tile as tile

    nc = bacc.Bacc("TRN2", target_bir_lowering=False, debug=False,
                   num_devices=8)
    with tile.TileContext(nc) as tc:
        _build(nc, tc, cfg, fl)
    nc.compile()
    return nc


def prep_inputs(inputs, cfg):
    """Host-side data prep. Returns (in_maps, fl)."""
    import ml_dtypes

    B, S, D, H, DFF, W, NT = (cfg.B, cfg.S, cfg.D, cfg.H, cfg.DFF,
                              cfg.W, cfg.NT)
    f = np.float32
    bf = ml_dtypes.bfloat16
    x = np.asarray(inputs["x"], f)
    enc = np.asarray(inputs["enc_out"])
    trg = np.asarray(inputs["trg_mask"])
    fl = Flags()
    fl.qkb1 = bool(np.any(inputs["qkv_b1"]))
    fl.qkb2 = bool(np.any(inputs["qkv_b2"]))
    fl.vb1 = bool(np.any(np.asarray(inputs["qkv_b1"])[2 * D :]))
    fl.vb2 = bool(np.any(np.asarray(inputs["qkv_b2"])[2 * D :]))
    fl.ob1 = bool(np.any(inputs["out_b1"]))
    fl.ob2 = bool(np.any(inputs["out_b2"]))
    fl.fb1 = bool(np.any(inputs["ff_b1"]))
    fl.fb2 = bool(np.any(inputs["ff_b2"]))
    fl.g1 = not bool(np.all(np.asarray(inputs["ln1_g"]) == 1))
    fl.b1 = bool(np.any(inputs["ln1_b"]))
    fl.g2 = not bool(np.all(np.asarray(inputs["ln2_g"]) == 1))
    fl.b2 = bool(np.any(inputs["ln2_b"]))
    fl.g3 = not bool(np.all(np.asarray(inputs["ln3_g"]) == 1))
    fl.b3 = bool(np.any(inputs["ln3_b"]))
    fl.m1 = not bool(np.all(trg != 0))
    fl.kb2 = bool(np.any(enc == 0))
    # causal <=> no mask entries above the diagonal (so key > query can be
    # skipped statically); queries are interleaved (core r gets q = r::4)
    # which makes the per-column needed-key count uniform across cores.
    if fl.m1:
        # trg[b, 0, q, k]: entries with k > q are the strictly-upper triangle
        fl.causal = not bool(np.any(np.triu(trg[:, 0], 1)))

    shared = {
        "qkvwT1": np.ascontiguousarray(np.asarray(inputs["qkv_w1"], f).T).astype(bf),
        "qkvwT2": np.ascontiguousarray(np.asarray(inputs["qkv_w2"], f).T).astype(bf),
        "owT1": np.ascontiguousarray(np.asarray(inputs["out_w1"], f).T).astype(bf),
        "owT2": np.ascontiguousarray(np.asarray(inputs["out_w2"], f).T).astype(bf),
        "w1T": np.ascontiguousarray(np.asarray(inputs["ff_w1"], f).T).astype(bf),
        "w2T": np.ascontiguousarray(np.asarray(inputs["ff_w2"], f).T).astype(bf),
    }
    if fl.qkb1:
        shared["qkvb1"] = np.asarray(inputs["qkv_b1"], f)
    if fl.qkb2:
        shared["qkvb2"] = np.asarray(inputs["qkv_b2"], f)
    if fl.vb1:
        shared["vb1"] = np.broadcast_to(
            np.asarray(inputs["qkv_b1"], f)[2 * D :], (P, D)).copy()
    if fl.vb2:
        shared["vb2"] = np.broadcast_to(
            np.asarray(inputs["qkv_b2"], f)[2 * D :], (P, D)).copy()
    if fl.ob1:
        shared["ob1"] = np.asarray(inputs["out_b1"], f)
    if fl.ob2:
        shared["ob2"] = np.asarray(inputs["out_b2"], f)
    if fl.fb1:
        shared["fb1"] = np.asarray(inputs["ff_b1"], f)
    if fl.fb2:
        shared["fb2"] = np.asarray(inputs["ff_b2"], f)
    for nm, key, use in [("g1", "ln1_g", fl.g1), ("b1", "ln1_b", fl.b1),
                         ("g2", "ln2_g", fl.g2), ("b2", "ln2_b", fl.b2),
                         ("g3", "ln3_g", fl.g3), ("b3", "ln3_b", fl.b3)]:
        if use:
            shared[nm] = np.asarray(inputs[key], f)

    xTb_ = [np.ascontiguousarray(x[b].T) for b in range(B)]
    xTbf = [t.astype(bf) for t in xTb_]
    # token held at gathered sequence position n = g*W + w is 4*w + g
    # (core g's local column w is global query 4w+g)
    tok_of_n = 4 * (np.arange(S) % W) + (np.arange(S) // W)
    in_maps = []
    for c in range(8):
        b, r = c // 4, c % 4
        qidx = np.arange(r, S, 4)  # this core's (interleaved) queries
        m = dict(shared)
        m["xTb"] = xTbf[b]
        m["xTlb"] = np.ascontiguousarray(xTbf[b][:, qidx])
        m["xTl"] = np.ascontiguousarray(xTb_[b][:, qidx])
        if fl.m1:
            # m1[kt, i, j] = trg[0or b, 0, qidx[j], kt*P + i]  (0/1)
            tb = trg[b] if trg.shape[0] == B else trg[0]
            blk = tb[0, qidx, :]  # [W, S] (q, k)
            m["m1"] = np.ascontiguousarray(
                (blk.T != 0).astype(bf).reshape(NT, P, W))
        if fl.kb2:
            eb = enc[b, 0, 0, :]  # [S], indexed by token
            kbv = np.where(eb[tok_of_n] != 0, f(0.0), f(-1e20)).astype(f)
            m["kb2"] = kbv.reshape(NT, P, 1)
        in_maps.append(m)
    return in_maps, fl


def kernel_with_results(_run_kwargs=None, **inputs):
    from concourse.bass_utils import run_bass_kernel_spmd

    cfg = Cfg()
    x = np.asarray(inputs["x"])
    assert x.shape == (cfg.B, cfg.S, cfg.D), x.shape
    in_maps, fl = prep_inputs(inputs, cfg)
    nc = make_program(cfg, fl)
    res = run_bass_kernel_spmd(nc, in_maps, list(range(8)),
                               **(_run_kwargs or {}))
    y = np.empty((cfg.B, cfg.S, cfg.D), np.float32)
    for c in range(8):
        b, r = c // 4, c % 4
        y[b, r::4, :] = res.results[c]["out"].T
    return y, res


def kernel(**inputs):
    return kernel_with_results(**inputs)[0]
